# revision 1
# baseline (speedup 1.0000x reference)
"""Multi-head self-attention (RoPE, eval-mode) Trainium2 Bass kernel.

Problem: B=2, T=2048, D=1024, H=16, d_head=64, fp32 I/O.

Sharding (8 cores): core c handles batch b=c//4 and the 4 heads
[4g, 4g+4) where g=c%4.  QKV/attention are head-local; the output
projection produces a per-core partial (contraction over this core's
256 head-dims) which the host sums across the 4 cores of each batch
and adds b_out.

Per-core design notes:
  - q,k are computed feature-major (d_head on partitions, T on free) so
    scores^T tiles come straight from matmuls; 2 heads stacked per
    128-partition tile, scores for both heads issued as row-packed
    (K=64) concurrent matmuls.
  - RoPE: rotate_half is a 32-partition block swap (SBUF->SBUF DMAs)
    with the sign folded into the host-provided sin table;
    q' = q*cos + rot(q)*sin_signed, with one mul on DVE, one on GpSimd.
    Rope work is emitted interleaved with remaining QKV matmuls so the
    PE never idles long enough for HAM to re-throttle the clock.
  - v is computed row-major [t, dv] and stored per head as [ones | v]
    128-wide stationary tiles, so each PV matmul yields the softmax
    denominators (partitions 0:64, replicated) and attn^T (64:128).
  - softmax skips max-subtraction (scores ~ N(0,1), exp safe in fp32)
    and normalizes after PV with the fast DVE reciprocal (base-0 only).
  - matmuls run as float32r (single-pass fp32 mode, full rate at
    moving dim >= 256).
  - attention uses tq=512 blocks: PSUM = 2 score tiles [128,1024]
    (both heads side by side, double buffered) + 2 PV accumulators
    [128,512] = 6 banks, one exp instr per tk tile.
"""

import numpy as np

B, T, D = 2, 2048, 1024
H = 16
DH = 64
NCORES = 8
P = 128

_CACHE = {}


def _rope_tables_np():
    theta = 1.0 / (10000.0 ** (np.arange(0, DH, 2, dtype=np.float32) / DH))
    angles = np.outer(np.arange(T, dtype=np.float32), theta)  # (T, 32)
    angles = np.concatenate([angles, angles], axis=-1)  # (T, DH)
    cos = np.cos(angles).astype(np.float32)
    sin = np.sin(angles).astype(np.float32)
    cosT = np.ascontiguousarray(cos.T)  # (64, T)
    sinT = np.ascontiguousarray(sin.T)
    sinT_signed = np.concatenate([-sinT[0:32], sinT[32:64]], axis=0)
    cos2 = np.tile(cosT, (2, 1))  # (128, T)
    sin2 = np.tile(sinT_signed, (2, 1))
    return cos2, sin2


def _build_module():
    import concourse.mybir as mybir
    import concourse.tile as tile
    from concourse import bacc

    f32 = mybir.dt.float32
    f32r = mybir.dt.float32r

    nc = bacc.Bacc("TRN2", target_bir_lowering=False, debug=False)
    xT = nc.dram_tensor("xT", [4, P, 8, 512], f32r, kind="ExternalInput")
    w_qk = nc.dram_tensor("w_qk", [P, 8, 512], f32r, kind="ExternalInput")
    w_v = nc.dram_tensor("w_v", [P, 8, 256], f32r, kind="ExternalInput")
    w_o = nc.dram_tensor("w_o", [P, 2, 1024], f32r, kind="ExternalInput")
    cos2 = nc.dram_tensor("cos2", [P, T], f32, kind="ExternalInput")
    sin2 = nc.dram_tensor("sin2", [P, T], f32, kind="ExternalInput")
    out = nc.dram_tensor("out", [T, D], f32, kind="ExternalOutput")

    Exp = mybir.ActivationFunctionType.Exp

    with tile.TileContext(nc) as tc:
        with tc.tile_pool(name="persist", bufs=1) as persist:
            wqk_sb = [
                persist.tile([P, 4, 512], f32r, tag=f"wqk{i}", name=f"wqk{i}")
                for i in range(2)
            ]
            wv_sb = persist.tile([P, 8, 256], f32r)
            # q_q[hp][qtr]: roped q, two heads stacked, per T-quarter.
            q_q = [
                [
                    persist.tile([P, 512], f32r, tag=f"q{hp}_{q}", name=f"q{hp}_{q}")
                    for q in range(4)
                ]
                for hp in range(2)
            ]
            # kpad[hp][h][qtr]: roped k per head, zero-padded to K=128 (head
            # 0 in rows 0:64, head 1 in rows 64:128, matching the stacked q
            # rhs).  All-K=128 matmuls keep the PE clock at full rate —
            # mixing row-grp K=64 mms with K=128 mms sticks HAM at 1.2 GHz.
            kpad = [
                [
                    [
                        persist.tile(
                            [P, 512], f32r, tag=f"kp{hp}{h}_{q}", name=f"kp{hp}{h}_{q}"
                        )
                        for q in range(4)
                    ]
                    for h in range(2)
                ]
                for hp in range(2)
            ]
            # per (tk-tile, head): [ones | v] stationary 128x128
            vaug = persist.tile([P, 16, 4, P], f32r)

            # Attention-phase SBUF pools open BEFORE the QKV pools so their
            # addresses never alias QKV workspace (an aliased exp output
            # would inherit a WAR dependency on the whole rope pipeline).
            with (
                tc.tile_pool(name="attnsb", bufs=1) as apool,
                tc.tile_pool(name="expp", bufs=3) as epool,
                tc.tile_pool(name="norm", bufs=1) as npool,
            ):
                attn_q = [
                    [
                        apool.tile(
                            [P, 512], f32r, tag=f"at{hp}_{b}", name=f"at{hp}_{b}"
                        )
                        for b in range(4)
                    ]
                    for hp in range(2)
                ]

                with (
                    tc.tile_pool(name="xt", bufs=2) as xpool,
                    tc.tile_pool(name="kst", bufs=1) as kpool,
                    tc.tile_pool(name="qkv_ps", bufs=2, space="PSUM") as qkps,
                    tc.tile_pool(name="rope", bufs=2) as rpool,
                ):
                    cos_sb = kpool.tile([P, T], f32)
                    sin_sb = kpool.tile([P, T], f32)
                    kstack = [
                        [
                            kpool.tile(
                                [P, 512], f32r, tag=f"ks{hp}_{q}", name=f"ks{hp}_{q}"
                            )
                            for q in range(4)
                        ]
                        for hp in range(2)
                    ]
                    for hp in range(2):
                        for q in range(4):
                            nc.vector.memset(
                                kpad[hp][0][q][64:128, :].bitcast(f32), 0.0
                            )
                            nc.vector.memset(
                                kpad[hp][1][q][0:64, :].bitcast(f32), 0.0
                            )

                    def fm_dst(cc, tq):
                        return (q_q if cc in (0, 2) else kstack)[cc // 2][tq]

                    def fm_chain(xt, tq, cc):
                        """One feature-major QKV chain (q or stacked k)."""
                        ps = qkps.tile([P, 512], f32, tag="fm", name="fmps")
                        for dc in range(8):
                            nc.tensor.matmul(
                                ps[:],
                                lhsT=wqk_sb[dc // 4][:, dc % 4, cc * P : (cc + 1) * P],
                                rhs=xt[dc // 4][:, dc % 4, :],
                                start=(dc == 0),
                                stop=(dc == 7),
                            )
                        nc.vector.tensor_copy(fm_dst(cc, tq)[:], ps[:])

                    def v_chain(xt, tq, t4):
                        psv = qkps.tile([P, 256], f32, tag="v", name="vps")
                        for dc in range(8):
                            nc.tensor.matmul(
                                psv[:],
                                lhsT=xt[dc // 4][:, dc % 4, t4 * P : (t4 + 1) * P],
                                rhs=wv_sb[:, dc, :],
                                start=(dc == 0),
                                stop=(dc == 7),
                            )
                        tki = tq * 4 + t4
                        nc.scalar.copy(
                            vaug[:, tki, :, 64:128],
                            psv.rearrange("p (h e) -> p h e", e=64),
                        )

                    def rope_q(cc, qtr):
                        """RoPE one T-quarter of one q/k tensor.  q is roped
                        in place; stacked k is roped into the per-head
                        zero-padded kpad tiles (two half-adds)."""
                        base = fm_dst(cc, qtr)
                        hs = slice(qtr * 512, (qtr + 1) * 512)
                        rot = rpool.tile([P, 512], f32, tag="rot", name="rot")
                        for blk in range(4):
                            s = (blk ^ 1) * 32
                            eng = nc.sync if blk % 2 == 0 else nc.gpsimd
                            eng.dma_start(
                                rot[blk * 32 : (blk + 1) * 32, :],
                                base[s : s + 32, :].bitcast(f32),
                            )
                        t1 = rpool.tile([P, 512], f32, tag="t1", name="t1")
                        nc.vector.tensor_mul(t1[:], base[:].bitcast(f32), cos_sb[:, hs])
                        nc.vector.tensor_mul(rot[:], rot[:], sin_sb[:, hs])
                        if cc in (0, 2):
                            nc.vector.tensor_add(base[:], t1[:], rot[:])
                        else:
                            hp = cc // 2
                            nc.vector.tensor_add(
                                kpad[hp][0][qtr][0:64, :], t1[0:64, :], rot[0:64, :]
                            )
                            nc.vector.tensor_add(
                                kpad[hp][1][qtr][64:128, :],
                                t1[64:128, :],
                                rot[64:128, :],
                            )

                    nc.scalar.dma_start(wqk_sb[0][:], w_qk[:, 0:4, :])
                    nc.gpsimd.dma_start(wqk_sb[1][:], w_qk[:, 4:8, :])
                    xts = []
                    for tq in range(4):
                        xtl = xpool.tile([P, 4, 512], f32r, tag="xtl", name="xtl")
                        xth = xpool.tile([P, 4, 512], f32r, tag="xth", name="xth")
                        nc.sync.dma_start(xtl[:], xT[tq, :, 0:4, :])
                        eng = nc.gpsimd if tq == 0 else nc.sync
                        eng.dma_start(xth[:], xT[tq, :, 4:8, :])
                        xts.append((xtl, xth))
                    nc.scalar.dma_start(wv_sb[:], w_v[:])
                    nc.vector.memset(vaug[:, :, :, 0:64].bitcast(f32), 1.0)
                    nc.scalar.dma_start(cos_sb[:], cos2[:])
                    nc.scalar.dma_start(sin_sb[:], sin2[:])

                    # Quarter-major; each quarter's ropes follow its chains
                    # so RoPE pipelines with QKV.
                    for tq in range(4):
                        for cc in (1, 0, 3, 2):
                            fm_chain(xts[tq], tq, cc)
                        for t4 in range(4):
                            v_chain(xts[tq], tq, t4)
                        for cc in (1, 0, 3, 2):
                            rope_q(cc, tq)

                # ---- attention + interleaved output projection ----------
                # Outproj for tq-block b is emitted right after attention
                # (hp1, b) so it hides under the remaining attention's ACT
                # time; PSUM: sc 4 + pv 2 + po 2 = 8 banks.
                with (
                    tc.tile_pool(name="wop", bufs=1) as wpool,
                    tc.tile_pool(name="ob", bufs=3) as opool,
                    tc.tile_pool(name="sc_ps", bufs=2, space="PSUM") as scps,
                    tc.tile_pool(name="pv_ps", bufs=1, space="PSUM") as pvps,
                    tc.tile_pool(name="po_ps", bufs=2, space="PSUM") as pops,
                ):
                    wo_sb = wpool.tile([P, 2, 1024], f32r)
                    nc.sync.dma_start(wo_sb[:], w_o[:])

                    def outproj_unit(b, tqc):
                        row = b * 4 + tqc
                        for d2 in range(2):
                            po = pops.tile([P, 512], f32, tag="po", name="po")
                            for hp in range(2):
                                nc.tensor.matmul(
                                    po[:],
                                    lhsT=attn_q[hp][b][:, tqc * P : (tqc + 1) * P],
                                    rhs=wo_sb[:, hp, d2 * 512 : (d2 + 1) * 512],
                                    start=(hp == 0),
                                    stop=(hp == 1),
                                )
                            ob = opool.tile([P, 512], f32, tag="ob", name="ob")
                            nc.vector.tensor_copy(ob[:], po[:])
                            seng = nc.sync if d2 == 0 else nc.gpsimd
                            seng.dma_start(
                                out[row * P : (row + 1) * P, d2 * 512 : (d2 + 1) * 512],
                                ob[:],
                            )

                    for hp in range(2):
                        for tq in range(4):  # tq blocks of 512
                            prev_b = tq - 1 if (hp == 1 and tq > 0) else None
                            pv = [
                                pvps.tile([P, 512], f32, tag=f"pv{h}", name=f"pv{h}")
                                for h in range(2)
                            ]
                            for tk in range(16):
                                if prev_b is not None and tk % 4 == 3:
                                    outproj_unit(prev_b, tk // 4)
                                sc = scps.tile([P, 1024], f32, tag="sc", name="sc")
                                ko = (tk % 4) * P
                                for h in range(2):
                                    nc.tensor.matmul(
                                        sc[:, h * 512 : (h + 1) * 512],
                                        lhsT=kpad[hp][h][tk // 4][:, ko : ko + P],
                                        rhs=q_q[hp][tq][:],
                                        start=True,
                                        stop=True,
                                    )
                                ex = epool.tile([P, 1024], f32r, tag="e", name="e")
                                nc.scalar.activation(ex[:], sc[:], Exp, scale=0.125)
                                for h in range(2):
                                    nc.tensor.matmul(
                                        pv[h][:],
                                        lhsT=vaug[:, tk, hp * 2 + h, :],
                                        rhs=ex[:, h * 512 : (h + 1) * 512],
                                        start=(tk == 0),
                                        stop=(tk == 15),
                                    )
                            for h in range(2):
                                rc = npool.tile([64, 512], f32, tag="rc", name="rc")
                                nc.vector.reciprocal_approx_fast(
                                    rc[:], pv[h][0:64, :]
                                )
                                hb = h * 64
                                nc.vector.tensor_mul(
                                    attn_q[hp][tq][hb : hb + 64, :],
                                    pv[h][64:128, :],
                                    rc[:],
                                )
                    for tqc in range(4):
                        outproj_unit(3, tqc)

    nc.compile()
    return nc


def _get_module():
    if "nc" not in _CACHE:
        _CACHE["nc"] = _build_module()
    return _CACHE["nc"]


def make_in_maps(x, w_qkv, w_out):
    cos2, sin2 = _rope_tables_np()
    in_maps = []
    for c in range(NCORES):
        b, g = divmod(c, 4)
        q0 = 256 * g
        # column chunks: [q_hp0 | k_hp0 | q_hp1 | k_hp1]
        wqk_c = np.concatenate(
            [
                w_qkv[:, q0 : q0 + 128],
                w_qkv[:, 1024 + q0 : 1024 + q0 + 128],
                w_qkv[:, q0 + 128 : q0 + 256],
                w_qkv[:, 1024 + q0 + 128 : 1024 + q0 + 256],
            ],
            axis=1,
        )
        xt4 = np.ascontiguousarray(
            x[b].T.reshape(8, 128, 4, 512).transpose(2, 1, 0, 3)
        )
        wv_c = w_qkv[:, 2048 + q0 : 2048 + q0 + 256]
        in_maps.append(
            {
                "xT": xt4,
                "w_qk": np.ascontiguousarray(
                    wqk_c.reshape(8, 128, 512).transpose(1, 0, 2)
                ),
                "w_v": np.ascontiguousarray(
                    wv_c.reshape(8, 128, 256).transpose(1, 0, 2)
                ),
                "w_o": np.ascontiguousarray(
                    w_out[q0 : q0 + 256, :].reshape(2, 128, 1024).transpose(1, 0, 2)
                ),
                "cos2": cos2,
                "sin2": sin2,
            }
        )
    return in_maps


def combine_outputs(results, b_out):
    out = np.empty((B, T, D), dtype=np.float32)
    for b in range(B):
        acc = results[4 * b]["out"].astype(np.float32).copy()
        for c in range(4 * b + 1, 4 * b + 4):
            acc += results[c]["out"]
        out[b] = acc + b_out[None, :]
    return out


def kernel(x, w_qkv, w_out, b_out, _trace=False, _tag=[0]):
    from concourse import bass_utils

    nc = _get_module()
    in_maps = make_in_maps(
        np.asarray(x, dtype=np.float32),
        np.asarray(w_qkv, dtype=np.float32),
        np.asarray(w_out, dtype=np.float32),
    )
    res = bass_utils.run_bass_kernel_spmd(
        nc, in_maps, core_ids=list(range(NCORES)), trace=_trace
    )
    if _trace:
        _CACHE["last_result"] = res
    return combine_outputs(res.results, np.asarray(b_out, dtype=np.float32))



# revision 34
# speedup vs baseline: 1.0126x; 1.0126x over previous
"""Multi-head self-attention (RoPE, eval-mode) Trainium2 Bass kernel.

Problem: B=2, T=2048, D=1024, H=16, d_head=64, fp32 I/O.

Sharding (8 cores): core c handles batch b=c//4 and the 4 heads
[4g, 4g+4) where g=c%4.  QKV/attention are head-local; the output
projection produces a per-core partial (contraction over this core's
256 head-dims) which the host sums across the 4 cores of each batch
and adds b_out.

v2 design (vs the two-phase baseline):
  - The ACT exp stream (4 heads * T^2 = 16.8M elems ~ 110us streaming)
    is the hard wall.  The kernel is restructured so exp starts ~16us
    in instead of ~81us: per-quarter rounds emit the K/V/Q projection
    chains interleaved with attention tk-chunks of the first four
    (hp, tq) blocks, which accumulate PV partials into SBUF so the two
    PSUM pv banks don't serialize in-flight blocks.
  - DMA priority: w_qkv chunk 0 + x quarter 0 are issued first and the
    rest in need order, so the first matmul fires at ~6us not ~22us.
  - Scores are issued as two concurrent K=64 row-group matmuls
    (head 0 rows 0:64, head 1 rows 64:128) instead of zero-padded
    K=128 - halves score cycles; RoPE applies in place on the stacked
    k tiles (kstack == stationary source).
  - Emission skews sc one tk ahead of pv so the exp stream stays dense
    across chunk/block boundaries; per-head norm frees pv banks early.
  - PSUM: sc 2x[128,1024] (4 banks) + pv0/pv1 (2) + work ring 2 = 8.
  - v is computed row-major and stored per head as [ones | v] 128-wide
    stationary tiles: each PV matmul yields softmax denominators
    (partitions 0:64) and attn^T (64:128) in one pass.
  - softmax skips max-subtraction (scores ~ N(0,1), exp safe in fp32)
    and normalizes with the fast DVE reciprocal.
"""

import numpy as np

B, T, D = 2, 2048, 1024
H = 16
DH = 64
NCORES = 8
P = 128

_CACHE = {}
_DBG = False  # debug build: DMA intermediates of block (0,2) to "dbg"
_SKEW = True  # emit sc one tk ahead of the pv flush (denser ACT stream)
_OPTAIL = False  # emit all outproj units at the tail (diagnostic)
_ALLCHUNK = False  # process dense blocks as 4-tk chunks too (diagnostic)


def _rope_tables_np():
    theta = 1.0 / (10000.0 ** (np.arange(0, DH, 2, dtype=np.float32) / DH))
    angles = np.outer(np.arange(T, dtype=np.float32), theta)  # (T, 32)
    angles = np.concatenate([angles, angles], axis=-1)  # (T, DH)
    cos = np.cos(angles).astype(np.float32)
    sin = np.sin(angles).astype(np.float32)
    cosT = np.ascontiguousarray(cos.T)  # (64, T)
    sinT = np.ascontiguousarray(sin.T)
    sinT_signed = np.concatenate([-sinT[0:32], sinT[32:64]], axis=0)
    cos2 = np.tile(cosT, (2, 1))  # (128, T)
    sin2 = np.tile(sinT_signed, (2, 1))
    return cos2, sin2


def _build_module():
    import concourse.mybir as mybir
    import concourse.tile as tile
    from concourse import bacc

    f32 = mybir.dt.float32
    f32r = mybir.dt.float32r
    bf16 = mybir.dt.bfloat16

    nc = bacc.Bacc("TRN2", target_bir_lowering=False, debug=False)
    xT = nc.dram_tensor("xT", [4, P, 8, 512], f32r, kind="ExternalInput")
    w_qk = nc.dram_tensor("w_qk", [P, 8, 512], f32r, kind="ExternalInput")
    w_v = nc.dram_tensor("w_v", [P, 8, 256], f32r, kind="ExternalInput")
    w_o = nc.dram_tensor("w_o", [P, 2, 1024], f32r, kind="ExternalInput")
    cos2 = nc.dram_tensor("cos2", [P, T], f32, kind="ExternalInput")
    sin2 = nc.dram_tensor("sin2", [P, T], f32, kind="ExternalInput")
    out = nc.dram_tensor("out", [T, D], f32, kind="ExternalOutput")
    dbg = (
        nc.dram_tensor("dbg", [20, P, 512], f32, kind="ExternalOutput")
        if _DBG
        else None
    )

    Exp = mybir.ActivationFunctionType.Exp

    # Blocks in completion order.  The first CHUNKED ones accumulate PV
    # into SBUF in 4-tk chunks paced by quarter availability; the rest
    # run dense (16 tk straight, PV resident in PSUM).
    CHUNKED = [(0, 0), (1, 0), (0, 1), (1, 1)]
    DENSE = [(0, 2), (1, 2), (0, 3), (1, 3)]

    with tile.TileContext(nc) as tc:
        with tc.tile_pool(name="persist", bufs=1) as persist:
            wqk_sb = [
                persist.tile([P, 4, 512], f32r, tag=f"wqk{i}", name=f"wqk{i}")
                for i in range(2)
            ]
            wv_sb = persist.tile([P, 8, 256], f32r)
            wo_sb = persist.tile([P, 2, 1024], f32r)
            cos_sb = persist.tile([P, T], f32)
            sin_sb = persist.tile([P, T], f32)
            # roped q / stacked roped k, two heads per 128-partition tile
            q_q = [
                [persist.tile([P, 512], bf16, tag=f"q{hp}_{t}", name=f"q{hp}_{t}")
                 for t in range(4)]
                for hp in range(2)
            ]
            # zero-padded roped k per head (head h real rows h*64:h*64+64,
            # other half zero) - K=128 score matmuls need no row groups
            kpad = [
                [
                    [persist.tile([P, 512], bf16, tag=f"k{hp}{h}_{t}",
                                  name=f"k{hp}{h}_{t}")
                     for t in range(4)]
                    for h in range(2)
                ]
                for hp in range(2)
            ]
            # per (tk-tile, head): [ones | v] stationary 128x128
            vaug = persist.tile([P, 16, 4, P], f32r)
            attn_q = [
                [persist.tile([P, 512], f32r, tag=f"at{hp}_{b}", name=f"at{hp}_{b}")
                 for b in range(4)]
                for hp in range(2)
            ]
            # SBUF PV accumulators for the chunked blocks, partition-
            # aligned with attn_q: acc_n rows h*64:(h+1)*64 = head h
            # numerator (attn^T), acc_d same layout for denominators.
            acc_n = {
                blk: persist.tile([P, 512], f32, tag=f"an{blk[0]}{blk[1]}",
                                  name=f"an{blk[0]}{blk[1]}")
                for blk in CHUNKED
            }
            acc_d = {
                blk: persist.tile([P, 512], f32, tag=f"ad{blk[0]}{blk[1]}",
                                  name=f"ad{blk[0]}{blk[1]}")
                for blk in CHUNKED
            }

            with (
                tc.tile_pool(name="xt", bufs=2) as xpool,
                tc.tile_pool(name="rope", bufs=2) as rpool,
                tc.tile_pool(name="expp", bufs=3) as epool,
                tc.tile_pool(name="ob", bufs=2) as opool,
                tc.tile_pool(name="norm", bufs=1) as npool,
                tc.tile_pool(name="sc_ps", bufs=2, space="PSUM") as scps,
                tc.tile_pool(name="pv_ps", bufs=1, space="PSUM") as pvps,
                tc.tile_pool(name="wk_ps", bufs=2, space="PSUM") as wkps,
            ):
                # ---- input DMAs in priority order --------------------
                nc.sync.dma_start(wqk_sb[0][:], w_qk[:, 0:4, :])
                nc.gpsimd.dma_start(wqk_sb[1][:], w_qk[:, 4:8, :])
                xts = []
                for tq in range(2):  # prefetch quarters 0,1; 2,3 in rounds
                    xtl = xpool.tile([P, 4, 512], f32r, tag="xtl", name="xtl")
                    xth = xpool.tile([P, 4, 512], f32r, tag="xth", name="xth")
                    nc.sync.dma_start(xtl[:], xT[tq, :, 0:4, :])
                    nc.gpsimd.dma_start(xth[:], xT[tq, :, 4:8, :])
                    xts.append((xtl, xth))
                hs0 = slice(0, 512)
                nc.scalar.dma_start(cos_sb[:, hs0], cos2[:, hs0])
                nc.scalar.dma_start(sin_sb[:, hs0], sin2[:, hs0])
                nc.scalar.dma_start(wv_sb[:], w_v[:])
                for tq in range(1, 4):
                    hs = slice(tq * 512, (tq + 1) * 512)
                    nc.scalar.dma_start(cos_sb[:, hs], cos2[:, hs])
                    nc.scalar.dma_start(sin_sb[:, hs], sin2[:, hs])
                nc.scalar.dma_start(wo_sb[:], w_o[:])
                nc.vector.memset(vaug[:, :, :, 0:64].bitcast(f32), 1.0)
                for hp in range(2):
                    for t in range(4):
                        nc.vector.memset(kpad[hp][0][t][64:128, :], 0.0)
                        nc.vector.memset(kpad[hp][1][t][0:64, :], 0.0)

                # ---- unit emitters ----------------------------------
                def fm_chain(xt, dst, cc, drain):
                    """q or stacked-k feature-major chain -> dst (SBUF).
                    cc: column chunk in wqk ([q_hp0 | k_hp0 | q_hp1 | k_hp1])."""
                    ps = wkps.tile([P, 512], f32, tag="wk", name="wk")
                    for dc in range(8):
                        nc.tensor.matmul(
                            ps[:],
                            lhsT=wqk_sb[dc // 4][:, dc % 4, cc * P : (cc + 1) * P],
                            rhs=xt[dc // 4][:, dc % 4, :],
                            start=(dc == 0),
                            stop=(dc == 7),
                        )
                    drain(dst[:], ps[:])

                def v_pair(xt, tq, half):
                    """two T-128 blocks of v for all 4 heads -> vaug."""
                    psv = wkps.tile([P, 512], f32, tag="wk", name="wkv")
                    for t4 in (2 * half, 2 * half + 1):
                        off = (t4 % 2) * 256
                        for dc in range(8):
                            nc.tensor.matmul(
                                psv[:, off : off + 256],
                                lhsT=xt[dc // 4][:, dc % 4, t4 * P : (t4 + 1) * P],
                                rhs=wv_sb[:, dc, :],
                                start=(dc == 0),
                                stop=(dc == 7),
                            )
                    tki = tq * 4 + 2 * half
                    nc.scalar.copy(
                        vaug[:, tki : tki + 2, :, 64:128],
                        psv.rearrange("p (t h e) -> p t h e", t=2, e=64),
                    )

                def rope_mats(base, tq):
                    hs = slice(tq * 512, (tq + 1) * 512)
                    rot = rpool.tile([P, 512], bf16, tag="rot", name="rot")
                    for blk in range(4):
                        s = (blk ^ 1) * 32
                        eng = nc.sync if blk % 2 == 0 else nc.gpsimd
                        eng.dma_start(
                            rot[blk * 32 : (blk + 1) * 32, :],
                            base[s : s + 32, :],
                        )
                    t1 = rpool.tile([P, 512], f32, tag="t1", name="t1")
                    nc.vector.tensor_mul(t1[:], base[:], cos_sb[:, hs])
                    nc.vector.tensor_mul(rot[:], rot[:], sin_sb[:, hs])
                    return t1, rot

                def rope(base, tq):
                    """RoPE in place on a [128,512] stacked bf16 tile."""
                    t1, rot = rope_mats(base, tq)
                    nc.vector.tensor_add(base[:], t1[:], rot[:])

                def rope_k(ktmp, hp, tq):
                    """RoPE stacked k into the per-head zero-padded tiles."""
                    t1, rot = rope_mats(ktmp, tq)
                    nc.vector.tensor_add(
                        kpad[hp][0][tq][0:64, :], t1[0:64, :], rot[0:64, :]
                    )
                    nc.vector.tensor_add(
                        kpad[hp][1][tq][64:128, :], t1[64:128, :], rot[64:128, :]
                    )

                # attention step machinery: sc is emitted one tk ahead of
                # the pv flush so the ACT exp stream stays dense.
                pending = []  # list of (hp, tq, tk, ex, pv_pair, start, stop)

                def flush_pending():
                    while pending:
                        emit_pv(*pending.pop(0))

                def emit_pv(hp, tq, tk, ex, pvp, start, stop):
                    for h in range(2):
                        nc.tensor.matmul(
                            pvp[h][:],
                            lhsT=vaug[:, tk, hp * 2 + h, :],
                            rhs=ex[:, h * 512 : (h + 1) * 512],
                            start=start,
                            stop=stop,
                        )

                cur_pv = {}  # blk -> [pv0, pv1] while a chunk is in flight

                def att_step(hp, tq, tk, start, stop):
                    blk = (hp, tq)
                    sc = scps.tile([P, 1024], f32, tag="sc", name="sc")
                    ko = (tk % 4) * P
                    for h in range(2):
                        nc.tensor.matmul(
                            sc[:, h * 512 : (h + 1) * 512],
                            lhsT=kpad[hp][h][tk // 4][:, ko : ko + P],
                            rhs=q_q[hp][tq][:],
                            start=True,
                            stop=True,
                        )
                    ex = epool.tile([P, 1024], f32r, tag="e", name="e")
                    nc.scalar.activation(ex[:], sc[:], Exp, scale=0.125)
                    if _DBG and (hp, tq, tk) == (1, 2, 0):
                        for i, tsrc in ((0, q_q[1][2]), (1, kpad[1][0][0])):
                            db = opool.tile([P, 512], f32, tag="ob", name="db")
                            nc.vector.tensor_copy(db[:], tsrc[:])
                            nc.sync.dma_start(dbg[i], db[:])
                        for i in range(2):
                            db = opool.tile([P, 512], f32, tag="ob", name="db")
                            nc.vector.tensor_copy(
                                db[:], sc[:, i * 512 : (i + 1) * 512]
                            )
                            nc.sync.dma_start(dbg[2 + i], db[:])
                        nc.sync.dma_start(dbg[4], ex[:, 0:512].bitcast(f32))
                        nc.sync.dma_start(dbg[5], ex[:, 512:1024].bitcast(f32))
                    if start:
                        cur_pv[blk] = [
                            pvps.tile([P, 512], f32, tag=f"pv{h}", name=f"pv{h}")
                            for h in range(2)
                        ]
                    if pending:
                        emit_pv(*pending.pop(0))
                    pending.append((hp, tq, tk, ex, cur_pv[blk], start, stop))
                    if not _SKEW:
                        flush_pending()

                def att_chunk(hp, tq, tkq):
                    """4-tk chunk of a CHUNKED block; PV -> SBUF accum."""
                    blk = (hp, tq)
                    for i in range(4):
                        att_step(hp, tq, tkq * 4 + i, start=(i == 0), stop=(i == 3))
                    flush_pending()
                    pvp = cur_pv.pop(blk)
                    an, ad = acc_n[blk], acc_d[blk]
                    for h in range(2):
                        hb = slice(h * 64, (h + 1) * 64)
                        if tkq == 0:
                            nc.vector.tensor_copy(an[hb, :], pvp[h][64:128, :])
                            nc.vector.tensor_copy(ad[hb, :], pvp[h][0:64, :])
                        else:
                            nc.vector.tensor_add(
                                an[hb, :], an[hb, :], pvp[h][64:128, :]
                            )
                            nc.vector.tensor_add(
                                ad[hb, :], ad[hb, :], pvp[h][0:64, :]
                            )

                def norm_post(hp, tq):
                    if _DBG:
                        nc.gpsimd.dma_start(
                            dbg[8 + 4 * hp + tq], attn_q[hp][tq][:].bitcast(f32)
                        )

                def norm_chunked(hp, tq):
                    blk = (hp, tq)
                    rc2 = npool.tile([P, 512], f32, tag="rc2", name="rc2")
                    # single full-tile recip: the custom-DVE op is base-0 only
                    nc.vector.reciprocal_approx_fast(rc2[:], acc_d[blk][:])
                    for h in range(2):
                        hb = slice(h * 64, (h + 1) * 64)
                        nc.vector.tensor_mul(
                            attn_q[hp][tq][hb, :], acc_n[blk][hb, :], rc2[hb, :]
                        )
                    norm_post(hp, tq)

                def norm_dense(hp, tq, pvp):
                    if _DBG:
                        db = opool.tile([P, 512], f32, tag="ob", name="db")
                        nc.vector.tensor_copy(db[:], pvp[0][:])
                        nc.sync.dma_start(dbg[16 + 2 * (tq - 2) + hp], db[:])
                    for h in range(2):
                        rc = npool.tile([64, 512], f32, tag="rc", name="rc")
                        nc.vector.reciprocal_approx_fast(rc[:], pvp[h][0:64, :])
                        hb = h * 64
                        nc.vector.tensor_mul(
                            attn_q[hp][tq][hb : hb + 64, :],
                            pvp[h][64:128, :],
                            rc[:],
                        )
                    norm_post(hp, tq)

                def outproj_unit(b, tqc):
                    row = b * 4 + tqc
                    for d2 in range(2):
                        po = wkps.tile([P, 512], f32, tag="wk", name="po")
                        for hp in range(2):
                            nc.tensor.matmul(
                                po[:],
                                lhsT=attn_q[hp][b][:, tqc * P : (tqc + 1) * P],
                                rhs=wo_sb[:, hp, d2 * 512 : (d2 + 1) * 512],
                                start=(hp == 0),
                                stop=(hp == 1),
                            )
                        ob = opool.tile([P, 512], f32, tag="ob", name="ob")
                        nc.vector.tensor_copy(ob[:], po[:])
                        seng = nc.sync if d2 == 0 else nc.gpsimd
                        seng.dma_start(
                            out[row * P : (row + 1) * P, d2 * 512 : (d2 + 1) * 512],
                            ob[:],
                        )

                def drain_act(dst, ps):
                    nc.scalar.copy(dst, ps)

                def drain_dve(dst, ps):
                    nc.vector.tensor_copy(dst, ps)

                # ---- phase A: per-quarter rounds ---------------------
                # round j: [prefetch xt j+2] KV(j) Q(j) + attention
                # chunks of the CHUNKED blocks paced by availability.
                ROUND_CHUNKS = [
                    [(0, 0, 0), (1, 0, 0)],
                    [(0, 0, 1), (1, 0, 1), (0, 1, 0), (1, 1, 0)],
                    [(0, 0, 2), (1, 0, 2), (0, 1, 1), (1, 1, 1)],
                    [(0, 0, 3), (1, 0, 3), (0, 1, 2), (1, 1, 2)],
                ]
                for j in range(4):
                    xt = xts[j]
                    flush_pending()
                    ktmps = []
                    for hp in range(2):
                        kt = rpool.tile([P, 512], bf16, tag="kt", name="kt")
                        fm_chain(xt, kt, 2 * hp + 1, drain_act)
                        ktmps.append(kt)
                    for hp in range(2):
                        rope_k(ktmps[hp], hp, j)
                    v_pair(xt, j, 0)
                    v_pair(xt, j, 1)
                    for hp in range(2):
                        fm_chain(xt, q_q[hp][j], 2 * hp, drain_dve)
                    for hp in range(2):
                        rope(q_q[hp][j], j)
                    if _DBG and j == 2:
                        pass
                    # prefetch x quarter j+2 into this round's ring slot;
                    # emitted after the last reader of x quarter j so the
                    # WAR dependency keeps the recycled buffer safe.
                    if j + 2 < 4:
                        xtl = xpool.tile([P, 4, 512], f32r, tag="xtl", name="xtl")
                        xth = xpool.tile([P, 4, 512], f32r, tag="xth", name="xth")
                        nc.sync.dma_start(xtl[:], xT[j + 2, :, 0:4, :])
                        nc.gpsimd.dma_start(xth[:], xT[j + 2, :, 4:8, :])
                        xts.append((xtl, xth))
                    for hp, tq, tkq in ROUND_CHUNKS[j]:
                        att_chunk(hp, tq, tkq)

                # ---- phase B: finish chunked, run dense blocks -------
                att_chunk(0, 1, 3)
                norm_chunked(0, 0)
                norm_chunked(0, 1)
                att_chunk(1, 1, 3)
                norm_chunked(1, 0)
                norm_chunked(1, 1)

                # outproj b is interleaved into the dense stream once
                # both its norms are done: OP(0)/OP(1) into dense blocks
                # 0/1, OP(2) into dense block 3, OP(3) at the tail.
                OP_SLOTS = {
                    0: [(0, 0), (0, 1), (0, 2), (0, 3)],
                    1: [(1, 0), (1, 1), (1, 2), (1, 3)],
                    2: [(3, 0), (3, 1), (3, 2), (3, 3)],
                }
                op_at = {}
                for bi, slots in OP_SLOTS.items():
                    for si, (dbi, tqc) in enumerate(slots):
                        op_at.setdefault(dbi, []).append((si * 4 + 3, bi, tqc))

                for dbi, (hp, tq) in enumerate(DENSE):
                    blk = (hp, tq)
                    ops = dict(
                        (tk, (bi, tqc)) for tk, bi, tqc in op_at.get(dbi, [])
                    )
                    if _ALLCHUNK:
                        # donate the finished chunked block's accumulators
                        donor = CHUNKED[dbi]
                        acc_n[blk], acc_d[blk] = acc_n[donor], acc_d[donor]
                        for tkq in range(4):
                            att_chunk(hp, tq, tkq)
                            if not _OPTAIL and tkq * 4 + 3 in ops:
                                bi, tqc = ops[tkq * 4 + 3]
                                outproj_unit(bi, tqc)
                        norm_chunked(hp, tq)
                        continue
                    for tk in range(16):
                        att_step(hp, tq, tk, start=(tk == 0), stop=(tk == 15))
                        if tk in ops and not _OPTAIL:
                            bi, tqc = ops[tk]
                            outproj_unit(bi, tqc)
                    flush_pending()
                    pvp = cur_pv.pop(blk)
                    norm_dense(hp, tq, pvp)
                    if _DBG and (hp, tq) == (0, 2):
                        nc.sync.dma_start(
                            dbg[7], attn_q[0][2][:].bitcast(f32)
                        )
                if _OPTAIL:
                    for bi in range(3):
                        for tqc in range(4):
                            outproj_unit(bi, tqc)
                for tqc in range(4):
                    outproj_unit(3, tqc)

    nc.compile()
    return nc


def _get_module():
    if "nc" not in _CACHE:
        _CACHE["nc"] = _build_module()
    return _CACHE["nc"]


def make_in_maps(x, w_qkv, w_out):
    cos2, sin2 = _rope_tables_np()
    in_maps = []
    for c in range(NCORES):
        b, g = divmod(c, 4)
        q0 = 256 * g
        # column chunks: [q_hp0 | k_hp0 | q_hp1 | k_hp1]
        wqk_c = np.concatenate(
            [
                w_qkv[:, q0 : q0 + 128],
                w_qkv[:, 1024 + q0 : 1024 + q0 + 128],
                w_qkv[:, q0 + 128 : q0 + 256],
                w_qkv[:, 1024 + q0 + 128 : 1024 + q0 + 256],
            ],
            axis=1,
        )
        xt4 = np.ascontiguousarray(
            x[b].T.reshape(8, 128, 4, 512).transpose(2, 1, 0, 3)
        )
        wv_c = w_qkv[:, 2048 + q0 : 2048 + q0 + 256]
        in_maps.append(
            {
                "xT": xt4,
                "w_qk": np.ascontiguousarray(
                    wqk_c.reshape(8, 128, 512).transpose(1, 0, 2)
                ),
                "w_v": np.ascontiguousarray(
                    wv_c.reshape(8, 128, 256).transpose(1, 0, 2)
                ),
                "w_o": np.ascontiguousarray(
                    w_out[q0 : q0 + 256, :].reshape(2, 128, 1024).transpose(1, 0, 2)
                ),
                "cos2": cos2,
                "sin2": sin2,
            }
        )
    return in_maps


def combine_outputs(results, b_out):
    out = np.empty((B, T, D), dtype=np.float32)
    for b in range(B):
        acc = results[4 * b]["out"].astype(np.float32).copy()
        for c in range(4 * b + 1, 4 * b + 4):
            acc += results[c]["out"]
        out[b] = acc + b_out[None, :]
    return out


def kernel(x, w_qkv, w_out, b_out, _trace=False, _tag=[0]):
    from concourse import bass_utils

    nc = _get_module()
    in_maps = make_in_maps(
        np.asarray(x, dtype=np.float32),
        np.asarray(w_qkv, dtype=np.float32),
        np.asarray(w_out, dtype=np.float32),
    )
    res = bass_utils.run_bass_kernel_spmd(
        nc, in_maps, core_ids=list(range(NCORES)), trace=_trace
    )
    if _trace:
        _CACHE["last_result"] = res
    return combine_outputs(res.results, np.asarray(b_out, dtype=np.float32))


# revision 35
# speedup vs baseline: 1.1100x; 1.0961x over previous
"""Multi-head self-attention (RoPE, eval-mode) Trainium2 Bass kernel.

Problem: B=2, T=2048, D=1024, H=16, d_head=64, fp32 I/O.

Sharding (8 cores): core c handles batch b=c//4 and the 4 heads
[4g, 4g+4) where g=c%4.  QKV/attention are head-local; the output
projection produces a per-core partial (contraction over this core's
256 head-dims) which the host sums across the 4 cores of each batch
and adds b_out.

v2 design (vs the two-phase baseline):
  - The ACT exp stream (4 heads * T^2 = 16.8M elems ~ 110us streaming)
    is the hard wall.  The kernel is restructured so exp starts ~16us
    in instead of ~81us: per-quarter rounds emit the K/V/Q projection
    chains interleaved with attention tk-chunks of the first four
    (hp, tq) blocks, which accumulate PV partials into SBUF so the two
    PSUM pv banks don't serialize in-flight blocks.
  - DMA priority: w_qkv chunk 0 + x quarter 0 are issued first and the
    rest in need order, so the first matmul fires at ~6us not ~22us.
  - Scores are issued as two concurrent K=64 row-group matmuls
    (head 0 rows 0:64, head 1 rows 64:128) instead of zero-padded
    K=128 - halves score cycles; RoPE applies in place on the stacked
    k tiles (kstack == stationary source).
  - Emission skews sc one tk ahead of pv so the exp stream stays dense
    across chunk/block boundaries; per-head norm frees pv banks early.
  - PSUM: sc 2x[128,1024] (4 banks) + pv0/pv1 (2) + work ring 2 = 8.
  - v is computed row-major and stored per head as [ones | v] 128-wide
    stationary tiles: each PV matmul yields softmax denominators
    (partitions 0:64) and attn^T (64:128) in one pass.
  - softmax skips max-subtraction (scores ~ N(0,1), exp safe in fp32)
    and normalizes with the fast DVE reciprocal.
"""

import ml_dtypes
import numpy as np

BF16 = ml_dtypes.bfloat16

B, T, D = 2, 2048, 1024
H = 16
DH = 64
NCORES = 8
P = 128

_CACHE = {}
_DBG = False  # debug build: DMA intermediates of block (0,2) to "dbg"
_SKEW = True  # emit sc one tk ahead of the pv flush (denser ACT stream)
_OPTAIL = False  # emit all outproj units at the tail (diagnostic)
_ALLCHUNK = False  # process dense blocks as 4-tk chunks too (diagnostic)


def _rope_tables_np():
    theta = 1.0 / (10000.0 ** (np.arange(0, DH, 2, dtype=np.float32) / DH))
    angles = np.outer(np.arange(T, dtype=np.float32), theta)  # (T, 32)
    angles = np.concatenate([angles, angles], axis=-1)  # (T, DH)
    cos = np.cos(angles).astype(np.float32)
    sin = np.sin(angles).astype(np.float32)
    cosT = np.ascontiguousarray(cos.T)  # (64, T)
    sinT = np.ascontiguousarray(sin.T)
    sinT_signed = np.concatenate([-sinT[0:32], sinT[32:64]], axis=0)
    cos2 = np.tile(cosT, (2, 1))  # (128, T)
    sin2 = np.tile(sinT_signed, (2, 1))
    return cos2, sin2


def _build_module():
    import concourse.mybir as mybir
    import concourse.tile as tile
    from concourse import bacc

    f32 = mybir.dt.float32
    f32r = mybir.dt.float32r
    bf16 = mybir.dt.bfloat16

    nc = bacc.Bacc("TRN2", target_bir_lowering=False, debug=False)
    xT = nc.dram_tensor("xT", [4, P, 8, 512], bf16, kind="ExternalInput")
    w_qk = nc.dram_tensor("w_qk", [P, 8, 512], bf16, kind="ExternalInput")
    w_v = nc.dram_tensor("w_v", [P, 8, 256], bf16, kind="ExternalInput")
    w_o = nc.dram_tensor("w_o", [P, 2, 1024], f32r, kind="ExternalInput")
    cos2 = nc.dram_tensor("cos2", [P, T], bf16, kind="ExternalInput")
    sin2 = nc.dram_tensor("sin2", [P, T], bf16, kind="ExternalInput")
    out = nc.dram_tensor("out", [T, D], f32, kind="ExternalOutput")
    dbg = (
        nc.dram_tensor("dbg", [20, P, 512], f32, kind="ExternalOutput")
        if _DBG
        else None
    )

    Exp = mybir.ActivationFunctionType.Exp

    # Blocks in completion order.  The first CHUNKED ones accumulate PV
    # into SBUF in 4-tk chunks paced by quarter availability; the rest
    # run dense (16 tk straight, PV resident in PSUM).
    CHUNKED = [(0, 0), (1, 0), (0, 1), (1, 1)]
    DENSE = [(0, 2), (1, 2), (0, 3), (1, 3)]

    with tile.TileContext(nc) as tc:
        with tc.tile_pool(name="persist", bufs=1) as persist:
            wqk_sb = [
                persist.tile([P, 4, 512], bf16, tag=f"wqk{i}", name=f"wqk{i}")
                for i in range(2)
            ]
            wv_sb = persist.tile([P, 8, 256], bf16)
            wo_sb = persist.tile([P, 2, 1024], f32r)
            cos_sb = persist.tile([P, T], bf16)
            sin_sb = persist.tile([P, T], bf16)
            # roped q / stacked roped k, two heads per 128-partition tile
            q_q = [
                [persist.tile([P, 512], bf16, tag=f"q{hp}_{t}", name=f"q{hp}_{t}")
                 for t in range(4)]
                for hp in range(2)
            ]
            # zero-padded roped k per head (head h real rows h*64:h*64+64,
            # other half zero) - K=128 score matmuls need no row groups
            kpad = [
                [
                    [persist.tile([P, 512], bf16, tag=f"k{hp}{h}_{t}",
                                  name=f"k{hp}{h}_{t}")
                     for t in range(4)]
                    for h in range(2)
                ]
                for hp in range(2)
            ]
            # per (tk-tile, head): [ones | v] stationary 128x128
            vaug = persist.tile([P, 16, 4, P], bf16)
            attn_q = [
                [persist.tile([P, 512], f32r, tag=f"at{hp}_{b}", name=f"at{hp}_{b}")
                 for b in range(4)]
                for hp in range(2)
            ]
            # SBUF PV accumulators for the chunked blocks, partition-
            # aligned with attn_q: acc_n rows h*64:(h+1)*64 = head h
            # numerator (attn^T), acc_d same layout for denominators.
            acc_n = {
                blk: persist.tile([P, 512], f32, tag=f"an{blk[0]}{blk[1]}",
                                  name=f"an{blk[0]}{blk[1]}")
                for blk in CHUNKED
            }
            acc_d = {
                blk: persist.tile([P, 512], f32, tag=f"ad{blk[0]}{blk[1]}",
                                  name=f"ad{blk[0]}{blk[1]}")
                for blk in CHUNKED
            }

            with (
                tc.tile_pool(name="xt", bufs=2) as xpool,
                tc.tile_pool(name="rope", bufs=2) as rpool,
                tc.tile_pool(name="expp", bufs=4) as epool,
                tc.tile_pool(name="ob", bufs=2) as opool,
                tc.tile_pool(name="norm", bufs=1) as npool,
                tc.tile_pool(name="sc_ps", bufs=2, space="PSUM") as scps,
                tc.tile_pool(name="pv_ps", bufs=1, space="PSUM") as pvps,
                tc.tile_pool(name="wk_ps", bufs=2, space="PSUM") as wkps,
            ):
                # ---- input DMAs in priority order --------------------
                nc.sync.dma_start(wqk_sb[0][:], w_qk[:, 0:4, :])
                nc.gpsimd.dma_start(wqk_sb[1][:], w_qk[:, 4:8, :])
                xts = []
                for tq in range(2):  # prefetch quarters 0,1; 2,3 in rounds
                    xtl = xpool.tile([P, 4, 512], bf16, tag="xtl", name="xtl")
                    xth = xpool.tile([P, 4, 512], bf16, tag="xth", name="xth")
                    nc.sync.dma_start(xtl[:], xT[tq, :, 0:4, :])
                    nc.gpsimd.dma_start(xth[:], xT[tq, :, 4:8, :])
                    xts.append((xtl, xth))
                hs0 = slice(0, 512)
                nc.scalar.dma_start(cos_sb[:, hs0], cos2[:, hs0])
                nc.scalar.dma_start(sin_sb[:, hs0], sin2[:, hs0])
                nc.scalar.dma_start(wv_sb[:], w_v[:])
                for tq in range(1, 4):
                    hs = slice(tq * 512, (tq + 1) * 512)
                    nc.scalar.dma_start(cos_sb[:, hs], cos2[:, hs])
                    nc.scalar.dma_start(sin_sb[:, hs], sin2[:, hs])
                nc.scalar.dma_start(wo_sb[:], w_o[:])
                nc.vector.memset(vaug[:, :, :, 0:64], 1.0)
                for hp in range(2):
                    for t in range(4):
                        nc.vector.memset(kpad[hp][0][t][64:128, :], 0.0)
                        nc.vector.memset(kpad[hp][1][t][0:64, :], 0.0)

                # ---- unit emitters ----------------------------------
                def fm_chain(xt, dst, cc, drain):
                    """q or stacked-k feature-major chain -> dst (SBUF).
                    cc: column chunk in wqk ([q_hp0 | k_hp0 | q_hp1 | k_hp1])."""
                    ps = wkps.tile([P, 512], f32, tag="wk", name="wk")
                    for dc in range(8):
                        nc.tensor.matmul(
                            ps[:],
                            lhsT=wqk_sb[dc // 4][:, dc % 4, cc * P : (cc + 1) * P],
                            rhs=xt[dc // 4][:, dc % 4, :],
                            start=(dc == 0),
                            stop=(dc == 7),
                        )
                    drain(dst[:], ps[:])

                def v_pair(xt, tq, half):
                    """two T-128 blocks of v for all 4 heads -> vaug."""
                    psv = wkps.tile([P, 512], f32, tag="wk", name="wkv")
                    for t4 in (2 * half, 2 * half + 1):
                        off = (t4 % 2) * 256
                        for dc in range(8):
                            nc.tensor.matmul(
                                psv[:, off : off + 256],
                                lhsT=xt[dc // 4][:, dc % 4, t4 * P : (t4 + 1) * P],
                                rhs=wv_sb[:, dc, :],
                                start=(dc == 0),
                                stop=(dc == 7),
                            )
                    tki = tq * 4 + 2 * half
                    nc.scalar.copy(
                        vaug[:, tki : tki + 2, :, 64:128],
                        psv.rearrange("p (t h e) -> p t h e", t=2, e=64),
                    )

                def rope_mats(base, tq):
                    hs = slice(tq * 512, (tq + 1) * 512)
                    rot = rpool.tile([P, 512], bf16, tag="rot", name="rot")
                    for blk in range(4):
                        s = (blk ^ 1) * 32
                        eng = nc.sync if blk % 2 == 0 else nc.gpsimd
                        eng.dma_start(
                            rot[blk * 32 : (blk + 1) * 32, :],
                            base[s : s + 32, :],
                        )
                    t1 = rpool.tile([P, 512], bf16, tag="t1", name="t1")
                    nc.vector.tensor_mul(t1[:], base[:], cos_sb[:, hs])
                    nc.vector.tensor_mul(rot[:], rot[:], sin_sb[:, hs])
                    return t1, rot

                def rope(base, tq):
                    """RoPE in place on a [128,512] stacked bf16 tile."""
                    t1, rot = rope_mats(base, tq)
                    nc.vector.tensor_add(base[:], t1[:], rot[:])

                def rope_k(ktmp, hp, tq):
                    """RoPE stacked k into the per-head zero-padded tiles."""
                    t1, rot = rope_mats(ktmp, tq)
                    nc.vector.tensor_add(
                        kpad[hp][0][tq][0:64, :], t1[0:64, :], rot[0:64, :]
                    )
                    nc.vector.tensor_add(
                        kpad[hp][1][tq][64:128, :], t1[64:128, :], rot[64:128, :]
                    )

                # attention step machinery: sc is emitted one tk ahead of
                # the pv flush so the ACT exp stream stays dense.
                pending = []  # list of (hp, tq, tk, ex, pv_pair, start, stop)

                def flush_pending():
                    while pending:
                        emit_pv(*pending.pop(0))

                def emit_pv(hp, tq, tk, ex, pvp, start, stop):
                    for h in range(2):
                        nc.tensor.matmul(
                            pvp[h][:],
                            lhsT=vaug[:, tk, hp * 2 + h, :],
                            rhs=ex[:, h * 512 : (h + 1) * 512],
                            start=start,
                            stop=stop,
                        )

                cur_pv = {}  # blk -> [pv0, pv1] while a chunk is in flight

                def att_step(hp, tq, tk, start, stop):
                    blk = (hp, tq)
                    sc = scps.tile([P, 1024], f32, tag="sc", name="sc")
                    ko = (tk % 4) * P
                    for h in range(2):
                        nc.tensor.matmul(
                            sc[:, h * 512 : (h + 1) * 512],
                            lhsT=kpad[hp][h][tk // 4][:, ko : ko + P],
                            rhs=q_q[hp][tq][:],
                            start=True,
                            stop=True,
                        )
                    ex = epool.tile([P, 1024], bf16, tag="e", name="e")
                    nc.scalar.activation(ex[:], sc[:], Exp, scale=0.125)
                    if _DBG and (hp, tq, tk) == (1, 2, 0):
                        for i, tsrc in ((0, q_q[1][2]), (1, kpad[1][0][0])):
                            db = opool.tile([P, 512], f32, tag="ob", name="db")
                            nc.vector.tensor_copy(db[:], tsrc[:])
                            nc.sync.dma_start(dbg[i], db[:])
                        for i in range(2):
                            db = opool.tile([P, 512], f32, tag="ob", name="db")
                            nc.vector.tensor_copy(
                                db[:], sc[:, i * 512 : (i + 1) * 512]
                            )
                            nc.sync.dma_start(dbg[2 + i], db[:])
                        for i in range(2):
                            db = opool.tile([P, 512], f32, tag="ob", name="db")
                            nc.vector.tensor_copy(
                                db[:], ex[:, i * 512 : (i + 1) * 512]
                            )
                            nc.sync.dma_start(dbg[4 + i], db[:])
                    if start:
                        cur_pv[blk] = [
                            pvps.tile([P, 512], f32, tag=f"pv{h}", name=f"pv{h}")
                            for h in range(2)
                        ]
                    if pending:
                        emit_pv(*pending.pop(0))
                    pending.append((hp, tq, tk, ex, cur_pv[blk], start, stop))
                    if not _SKEW:
                        flush_pending()

                def att_chunk(hp, tq, tkq):
                    """4-tk chunk of a CHUNKED block; PV -> SBUF accum."""
                    blk = (hp, tq)
                    for i in range(4):
                        att_step(hp, tq, tkq * 4 + i, start=(i == 0), stop=(i == 3))
                    flush_pending()
                    pvp = cur_pv.pop(blk)
                    an, ad = acc_n[blk], acc_d[blk]
                    for h in range(2):
                        hb = slice(h * 64, (h + 1) * 64)
                        if tkq == 0:
                            nc.vector.tensor_copy(an[hb, :], pvp[h][64:128, :])
                            nc.vector.tensor_copy(ad[hb, :], pvp[h][0:64, :])
                        else:
                            nc.vector.tensor_add(
                                an[hb, :], an[hb, :], pvp[h][64:128, :]
                            )
                            nc.vector.tensor_add(
                                ad[hb, :], ad[hb, :], pvp[h][0:64, :]
                            )

                def norm_post(hp, tq):
                    if _DBG:
                        nc.gpsimd.dma_start(
                            dbg[8 + 4 * hp + tq], attn_q[hp][tq][:].bitcast(f32)
                        )

                def norm_chunked(hp, tq):
                    blk = (hp, tq)
                    rc2 = npool.tile([P, 512], f32, tag="rc2", name="rc2")
                    # single full-tile recip: the custom-DVE op is base-0 only
                    nc.vector.reciprocal_approx_fast(rc2[:], acc_d[blk][:])
                    for h in range(2):
                        hb = slice(h * 64, (h + 1) * 64)
                        nc.vector.tensor_mul(
                            attn_q[hp][tq][hb, :], acc_n[blk][hb, :], rc2[hb, :]
                        )
                    norm_post(hp, tq)

                def norm_dense(hp, tq, pvp):
                    if _DBG:
                        db = opool.tile([P, 512], f32, tag="ob", name="db")
                        nc.vector.tensor_copy(db[:], pvp[0][:])
                        nc.sync.dma_start(dbg[16 + 2 * (tq - 2) + hp], db[:])
                    for h in range(2):
                        rc = npool.tile([64, 512], f32, tag="rc", name="rc")
                        nc.vector.reciprocal_approx_fast(rc[:], pvp[h][0:64, :])
                        hb = h * 64
                        nc.vector.tensor_mul(
                            attn_q[hp][tq][hb : hb + 64, :],
                            pvp[h][64:128, :],
                            rc[:],
                        )
                    norm_post(hp, tq)

                def outproj_unit(b, tqc):
                    row = b * 4 + tqc
                    for d2 in range(2):
                        po = wkps.tile([P, 512], f32, tag="wk", name="po")
                        for hp in range(2):
                            nc.tensor.matmul(
                                po[:],
                                lhsT=attn_q[hp][b][:, tqc * P : (tqc + 1) * P],
                                rhs=wo_sb[:, hp, d2 * 512 : (d2 + 1) * 512],
                                start=(hp == 0),
                                stop=(hp == 1),
                            )
                        ob = opool.tile([P, 512], f32, tag="ob", name="ob")
                        if d2 == 0:
                            nc.vector.tensor_copy(ob[:], po[:])
                        else:
                            nc.scalar.copy(ob[:], po[:])
                        seng = nc.sync if d2 == 0 else nc.gpsimd
                        seng.dma_start(
                            out[row * P : (row + 1) * P, d2 * 512 : (d2 + 1) * 512],
                            ob[:],
                        )

                def drain_act(dst, ps):
                    nc.scalar.copy(dst, ps)

                def drain_dve(dst, ps):
                    nc.vector.tensor_copy(dst, ps)

                # ---- phase A: per-quarter rounds ---------------------
                # round j: [prefetch xt j+2] KV(j) Q(j) + attention
                # chunks of the CHUNKED blocks paced by availability.
                ROUND_CHUNKS = [
                    [(0, 0, 0), (1, 0, 0)],
                    [(0, 0, 1), (1, 0, 1), (0, 1, 0), (1, 1, 0)],
                    [(0, 0, 2), (1, 0, 2), (0, 1, 1), (1, 1, 1)],
                    [(0, 0, 3), (1, 0, 3), (0, 1, 2), (1, 1, 2)],
                ]
                for j in range(4):
                    xt = xts[j]
                    flush_pending()
                    ktmps = []
                    for hp in range(2):
                        kt = rpool.tile([P, 512], bf16, tag="kt", name="kt")
                        fm_chain(xt, kt, 2 * hp + 1, drain_act)
                        ktmps.append(kt)
                    for hp in range(2):
                        rope_k(ktmps[hp], hp, j)
                    v_pair(xt, j, 0)
                    v_pair(xt, j, 1)
                    for hp in range(2):
                        fm_chain(xt, q_q[hp][j], 2 * hp, drain_dve)
                    for hp in range(2):
                        rope(q_q[hp][j], j)
                    if _DBG and j == 2:
                        pass
                    # prefetch x quarter j+2 into this round's ring slot;
                    # emitted after the last reader of x quarter j so the
                    # WAR dependency keeps the recycled buffer safe.
                    if j + 2 < 4:
                        xtl = xpool.tile([P, 4, 512], bf16, tag="xtl", name="xtl")
                        xth = xpool.tile([P, 4, 512], bf16, tag="xth", name="xth")
                        nc.sync.dma_start(xtl[:], xT[j + 2, :, 0:4, :])
                        nc.gpsimd.dma_start(xth[:], xT[j + 2, :, 4:8, :])
                        xts.append((xtl, xth))
                    for hp, tq, tkq in ROUND_CHUNKS[j]:
                        att_chunk(hp, tq, tkq)

                # ---- phase B: finish chunked, run dense blocks -------
                att_chunk(0, 1, 3)
                norm_chunked(0, 0)
                norm_chunked(0, 1)
                att_chunk(1, 1, 3)
                norm_chunked(1, 0)
                norm_chunked(1, 1)

                # outproj b is interleaved into the dense stream once
                # both its norms are done: OP(0)/OP(1) into dense blocks
                # 0/1, OP(2) into dense block 3, OP(3) at the tail.
                OP_SLOTS = {
                    0: [(0, 0), (0, 1), (0, 2), (0, 3)],
                    1: [(1, 0), (1, 1), (1, 2), (1, 3)],
                    2: [(3, 0), (3, 1), (3, 2), (3, 3)],
                }
                op_at = {}
                for bi, slots in OP_SLOTS.items():
                    for si, (dbi, tqc) in enumerate(slots):
                        op_at.setdefault(dbi, []).append((si * 4 + 3, bi, tqc))

                for dbi, (hp, tq) in enumerate(DENSE):
                    blk = (hp, tq)
                    ops = dict(
                        (tk, (bi, tqc)) for tk, bi, tqc in op_at.get(dbi, [])
                    )
                    if _ALLCHUNK:
                        # donate the finished chunked block's accumulators
                        donor = CHUNKED[dbi]
                        acc_n[blk], acc_d[blk] = acc_n[donor], acc_d[donor]
                        for tkq in range(4):
                            att_chunk(hp, tq, tkq)
                            if not _OPTAIL and tkq * 4 + 3 in ops:
                                bi, tqc = ops[tkq * 4 + 3]
                                outproj_unit(bi, tqc)
                        norm_chunked(hp, tq)
                        continue
                    for tk in range(16):
                        att_step(hp, tq, tk, start=(tk == 0), stop=(tk == 15))
                        if tk in ops and not _OPTAIL:
                            bi, tqc = ops[tk]
                            outproj_unit(bi, tqc)
                    flush_pending()
                    pvp = cur_pv.pop(blk)
                    norm_dense(hp, tq, pvp)
                    if _DBG and (hp, tq) == (0, 2):
                        nc.sync.dma_start(
                            dbg[7], attn_q[0][2][:].bitcast(f32)
                        )
                if _OPTAIL:
                    for bi in range(3):
                        for tqc in range(4):
                            outproj_unit(bi, tqc)
                for tqc in range(4):
                    outproj_unit(3, tqc)

    nc.compile()
    return nc


def _get_module():
    if "nc" not in _CACHE:
        _CACHE["nc"] = _build_module()
    return _CACHE["nc"]


def make_in_maps(x, w_qkv, w_out):
    cos2, sin2 = _rope_tables_np()
    in_maps = []
    for c in range(NCORES):
        b, g = divmod(c, 4)
        q0 = 256 * g
        # column chunks: [q_hp0 | k_hp0 | q_hp1 | k_hp1]
        wqk_c = np.concatenate(
            [
                w_qkv[:, q0 : q0 + 128],
                w_qkv[:, 1024 + q0 : 1024 + q0 + 128],
                w_qkv[:, q0 + 128 : q0 + 256],
                w_qkv[:, 1024 + q0 + 128 : 1024 + q0 + 256],
            ],
            axis=1,
        )
        xt4 = np.ascontiguousarray(
            x[b].T.reshape(8, 128, 4, 512).transpose(2, 1, 0, 3)
        )
        wv_c = w_qkv[:, 2048 + q0 : 2048 + q0 + 256]
        in_maps.append(
            {
                "xT": xt4.astype(BF16),
                "w_qk": np.ascontiguousarray(
                    wqk_c.reshape(8, 128, 512).transpose(1, 0, 2)
                ).astype(BF16),
                "w_v": np.ascontiguousarray(
                    wv_c.reshape(8, 128, 256).transpose(1, 0, 2)
                ).astype(BF16),
                "w_o": np.ascontiguousarray(
                    w_out[q0 : q0 + 256, :].reshape(2, 128, 1024).transpose(1, 0, 2)
                ),
                "cos2": cos2.astype(BF16),
                "sin2": sin2.astype(BF16),
            }
        )
    return in_maps


def combine_outputs(results, b_out):
    out = np.empty((B, T, D), dtype=np.float32)
    for b in range(B):
        acc = results[4 * b]["out"].astype(np.float32).copy()
        for c in range(4 * b + 1, 4 * b + 4):
            acc += results[c]["out"]
        out[b] = acc + b_out[None, :]
    return out


def kernel(x, w_qkv, w_out, b_out, _trace=False, _tag=[0]):
    from concourse import bass_utils

    nc = _get_module()
    in_maps = make_in_maps(
        np.asarray(x, dtype=np.float32),
        np.asarray(w_qkv, dtype=np.float32),
        np.asarray(w_out, dtype=np.float32),
    )
    res = bass_utils.run_bass_kernel_spmd(
        nc, in_maps, core_ids=list(range(NCORES)), trace=_trace
    )
    if _trace:
        _CACHE["last_result"] = res
    return combine_outputs(res.results, np.asarray(b_out, dtype=np.float32))


# revision 36
# speedup vs baseline: 1.1191x; 1.0082x over previous
"""Multi-head self-attention (RoPE, eval-mode) Trainium2 Bass kernel.

Problem: B=2, T=2048, D=1024, H=16, d_head=64, fp32 I/O.

Sharding (8 cores): core c handles batch b=c//4 and the 4 heads
[4g, 4g+4) where g=c%4.  QKV/attention are head-local; the output
projection produces a per-core partial (contraction over this core's
256 head-dims) which the host sums across the 4 cores of each batch
and adds b_out.

v2 design (vs the two-phase baseline):
  - The ACT exp stream (4 heads * T^2 = 16.8M elems ~ 110us streaming)
    is the hard wall.  The kernel is restructured so exp starts ~16us
    in instead of ~81us: per-quarter rounds emit the K/V/Q projection
    chains interleaved with attention tk-chunks of the first four
    (hp, tq) blocks, which accumulate PV partials into SBUF so the two
    PSUM pv banks don't serialize in-flight blocks.
  - DMA priority: w_qkv chunk 0 + x quarter 0 are issued first and the
    rest in need order, so the first matmul fires at ~6us not ~22us.
  - Scores are issued as two concurrent K=64 row-group matmuls
    (head 0 rows 0:64, head 1 rows 64:128) instead of zero-padded
    K=128 - halves score cycles; RoPE applies in place on the stacked
    k tiles (kstack == stationary source).
  - Emission skews sc one tk ahead of pv so the exp stream stays dense
    across chunk/block boundaries; per-head norm frees pv banks early.
  - PSUM: sc 2x[128,1024] (4 banks) + pv0/pv1 (2) + work ring 2 = 8.
  - v is computed row-major and stored per head as [ones | v] 128-wide
    stationary tiles: each PV matmul yields softmax denominators
    (partitions 0:64) and attn^T (64:128) in one pass.
  - softmax skips max-subtraction (scores ~ N(0,1), exp safe in fp32)
    and normalizes with the fast DVE reciprocal.
"""

import ml_dtypes
import numpy as np

BF16 = ml_dtypes.bfloat16

B, T, D = 2, 2048, 1024
H = 16
DH = 64
NCORES = 8
P = 128

_CACHE = {}
_DBG = False  # debug build: DMA intermediates of block (0,2) to "dbg"
_SKEW = True  # emit sc one tk ahead of the pv flush (denser ACT stream)
_OPTAIL = False  # emit all outproj units at the tail (diagnostic)
_ALLCHUNK = False  # process dense blocks as 4-tk chunks too (diagnostic)


def _rope_tables_np():
    theta = 1.0 / (10000.0 ** (np.arange(0, DH, 2, dtype=np.float32) / DH))
    angles = np.outer(np.arange(T, dtype=np.float32), theta)  # (T, 32)
    angles = np.concatenate([angles, angles], axis=-1)  # (T, DH)
    cos = np.cos(angles).astype(np.float32)
    sin = np.sin(angles).astype(np.float32)
    cosT = np.ascontiguousarray(cos.T)  # (64, T)
    sinT = np.ascontiguousarray(sin.T)
    sinT_signed = np.concatenate([-sinT[0:32], sinT[32:64]], axis=0)
    cos2 = np.tile(cosT, (2, 1))  # (128, T)
    sin2 = np.tile(sinT_signed, (2, 1))
    return cos2, sin2


def _build_module():
    import concourse.mybir as mybir
    import concourse.tile as tile
    from concourse import bacc

    f32 = mybir.dt.float32
    f32r = mybir.dt.float32r
    bf16 = mybir.dt.bfloat16

    nc = bacc.Bacc("TRN2", target_bir_lowering=False, debug=False)
    xT = nc.dram_tensor("xT", [4, P, 8, 512], bf16, kind="ExternalInput")
    w_qk = nc.dram_tensor("w_qk", [P, 8, 512], bf16, kind="ExternalInput")
    w_v = nc.dram_tensor("w_v", [P, 8, 256], bf16, kind="ExternalInput")
    w_o = nc.dram_tensor("w_o", [P, 2, 1024], f32r, kind="ExternalInput")
    cos2 = nc.dram_tensor("cos2", [P, T], bf16, kind="ExternalInput")
    sin2 = nc.dram_tensor("sin2", [P, T], bf16, kind="ExternalInput")
    out = nc.dram_tensor("out", [T, D], f32, kind="ExternalOutput")
    dbg = (
        nc.dram_tensor("dbg", [20, P, 512], f32, kind="ExternalOutput")
        if _DBG
        else None
    )

    Exp = mybir.ActivationFunctionType.Exp

    # Blocks in completion order.  The first CHUNKED ones accumulate PV
    # into SBUF in 4-tk chunks paced by quarter availability; the rest
    # run dense (16 tk straight, PV resident in PSUM).
    CHUNKED = [(hp, tq) for tq in range(4) for hp in range(2)]

    with tile.TileContext(nc) as tc:
        with tc.tile_pool(name="persist", bufs=1) as persist:
            wqk_sb = [
                persist.tile([P, 4, 512], bf16, tag=f"wqk{i}", name=f"wqk{i}")
                for i in range(2)
            ]
            wv_sb = persist.tile([P, 8, 256], bf16)
            wo_sb = persist.tile([P, 2, 1024], f32r)
            cos_sb = persist.tile([P, T], bf16)
            sin_sb = persist.tile([P, T], bf16)
            # roped q / stacked roped k, two heads per 128-partition tile
            q_q = [
                [persist.tile([P, 512], bf16, tag=f"q{hp}_{t}", name=f"q{hp}_{t}")
                 for t in range(4)]
                for hp in range(2)
            ]
            # zero-padded roped k per head (head h real rows h*64:h*64+64,
            # other half zero) - K=128 score matmuls need no row groups
            kpad = [
                [
                    [persist.tile([P, 512], bf16, tag=f"k{hp}{h}_{t}",
                                  name=f"k{hp}{h}_{t}")
                     for t in range(4)]
                    for h in range(2)
                ]
                for hp in range(2)
            ]
            # per (tk-tile, head): [ones | v] stationary 128x128
            vaug = persist.tile([P, 16, 4, P], bf16)
            attn_q = [
                [persist.tile([P, 512], f32r, tag=f"at{hp}_{b}", name=f"at{hp}_{b}")
                 for b in range(4)]
                for hp in range(2)
            ]
            # SBUF PV accumulators for the chunked blocks, partition-
            # aligned with attn_q: acc_n rows h*64:(h+1)*64 = head h
            # numerator (attn^T), acc_d same layout for denominators.
            acc_n = {
                blk: persist.tile([P, 512], f32, tag=f"an{blk[0]}{blk[1]}",
                                  name=f"an{blk[0]}{blk[1]}")
                for blk in CHUNKED
            }
            acc_d = {
                blk: persist.tile([P, 512], f32, tag=f"ad{blk[0]}{blk[1]}",
                                  name=f"ad{blk[0]}{blk[1]}")
                for blk in CHUNKED
            }

            with (
                tc.tile_pool(name="xt", bufs=2) as xpool,
                tc.tile_pool(name="rope", bufs=2) as rpool,
                tc.tile_pool(name="expp", bufs=4) as epool,
                tc.tile_pool(name="ob", bufs=4) as opool,
                tc.tile_pool(name="norm", bufs=1) as npool,
                tc.tile_pool(name="sc_ps", bufs=2, space="PSUM") as scps,
                tc.tile_pool(name="pv_ps", bufs=1, space="PSUM") as pvps,
                tc.tile_pool(name="wk_ps", bufs=2, space="PSUM") as wkps,
            ):
                # ---- input DMAs in priority order --------------------
                nc.sync.dma_start(wqk_sb[0][:], w_qk[:, 0:4, :])
                nc.gpsimd.dma_start(wqk_sb[1][:], w_qk[:, 4:8, :])
                xts = []
                xtl = xpool.tile([P, 4, 512], bf16, tag="xtl", name="xtl")
                xth = xpool.tile([P, 4, 512], bf16, tag="xth", name="xth")
                nc.sync.dma_start(xtl[:], xT[0, :, 0:4, :])
                nc.gpsimd.dma_start(xth[:], xT[0, :, 4:8, :])
                xts.append((xtl, xth))
                hs0 = slice(0, 512)
                nc.scalar.dma_start(cos_sb[:, hs0], cos2[:, hs0])
                nc.scalar.dma_start(sin_sb[:, hs0], sin2[:, hs0])
                nc.scalar.dma_start(wv_sb[:], w_v[:])
                nc.vector.memset(vaug[:, :, :, 0:64], 1.0)
                for hp in range(2):
                    for t in range(4):
                        nc.vector.memset(kpad[hp][0][t][64:128, :], 0.0)
                        nc.vector.memset(kpad[hp][1][t][0:64, :], 0.0)

                # ---- unit emitters ----------------------------------
                def fm_chain(xt, dst, cc, drain):
                    """q or stacked-k feature-major chain -> dst (SBUF).
                    cc: column chunk in wqk ([q_hp0 | k_hp0 | q_hp1 | k_hp1])."""
                    ps = wkps.tile([P, 512], f32, tag="wk", name="wk")
                    for dc in range(8):
                        nc.tensor.matmul(
                            ps[:],
                            lhsT=wqk_sb[dc // 4][:, dc % 4, cc * P : (cc + 1) * P],
                            rhs=xt[dc // 4][:, dc % 4, :],
                            start=(dc == 0),
                            stop=(dc == 7),
                        )
                    drain(dst[:], ps[:])

                def v_pair(xt, tq, half):
                    """two T-128 blocks of v for all 4 heads -> vaug."""
                    psv = wkps.tile([P, 512], f32, tag="wk", name="wkv")
                    for t4 in (2 * half, 2 * half + 1):
                        off = (t4 % 2) * 256
                        for dc in range(8):
                            nc.tensor.matmul(
                                psv[:, off : off + 256],
                                lhsT=xt[dc // 4][:, dc % 4, t4 * P : (t4 + 1) * P],
                                rhs=wv_sb[:, dc, :],
                                start=(dc == 0),
                                stop=(dc == 7),
                            )
                    tki = tq * 4 + 2 * half
                    nc.scalar.copy(
                        vaug[:, tki : tki + 2, :, 64:128],
                        psv.rearrange("p (t h e) -> p t h e", t=2, e=64),
                    )

                def rope_mats(base, tq):
                    hs = slice(tq * 512, (tq + 1) * 512)
                    rot = rpool.tile([P, 512], bf16, tag="rot", name="rot")
                    for blk in range(4):
                        s = (blk ^ 1) * 32
                        eng = nc.sync if blk % 2 == 0 else nc.gpsimd
                        eng.dma_start(
                            rot[blk * 32 : (blk + 1) * 32, :],
                            base[s : s + 32, :],
                        )
                    t1 = rpool.tile([P, 512], bf16, tag="t1", name="t1")
                    nc.vector.tensor_mul(t1[:], base[:], cos_sb[:, hs])
                    nc.vector.tensor_mul(rot[:], rot[:], sin_sb[:, hs])
                    return t1, rot

                def rope(base, tq):
                    """RoPE in place on a [128,512] stacked bf16 tile."""
                    t1, rot = rope_mats(base, tq)
                    nc.vector.tensor_add(base[:], t1[:], rot[:])

                def rope_k(ktmp, hp, tq):
                    """RoPE stacked k into the per-head zero-padded tiles."""
                    t1, rot = rope_mats(ktmp, tq)
                    nc.vector.tensor_add(
                        kpad[hp][0][tq][0:64, :], t1[0:64, :], rot[0:64, :]
                    )
                    nc.vector.tensor_add(
                        kpad[hp][1][tq][64:128, :], t1[64:128, :], rot[64:128, :]
                    )

                # attention step machinery: sc is emitted one tk ahead of
                # the pv flush so the ACT exp stream stays dense.
                pending = []  # list of (hp, tq, tk, ex, pv_pair, start, stop)

                def flush_pending():
                    while pending:
                        emit_pv(*pending.pop(0))

                def emit_pv(hp, tq, tk, ex, pvp, start, stop):
                    for h in range(2):
                        nc.tensor.matmul(
                            pvp[h][:],
                            lhsT=vaug[:, tk, hp * 2 + h, :],
                            rhs=ex[:, h * 512 : (h + 1) * 512],
                            start=start,
                            stop=stop,
                        )

                cur_pv = {}  # blk -> [pv0, pv1] while a chunk is in flight

                def att_step(hp, tq, tk, start, stop):
                    blk = (hp, tq)
                    sc = scps.tile([P, 1024], f32, tag="sc", name="sc")
                    ko = (tk % 4) * P
                    for h in range(2):
                        nc.tensor.matmul(
                            sc[:, h * 512 : (h + 1) * 512],
                            lhsT=kpad[hp][h][tk // 4][:, ko : ko + P],
                            rhs=q_q[hp][tq][:],
                            start=True,
                            stop=True,
                        )
                    ex = epool.tile([P, 1024], bf16, tag="e", name="e")
                    nc.scalar.activation(ex[:], sc[:], Exp, scale=0.125)
                    if _DBG and (hp, tq, tk) == (1, 2, 0):
                        for i, tsrc in ((0, q_q[1][2]), (1, kpad[1][0][0])):
                            db = opool.tile([P, 512], f32, tag="ob", name="db")
                            nc.vector.tensor_copy(db[:], tsrc[:])
                            nc.sync.dma_start(dbg[i], db[:])
                        for i in range(2):
                            db = opool.tile([P, 512], f32, tag="ob", name="db")
                            nc.vector.tensor_copy(
                                db[:], sc[:, i * 512 : (i + 1) * 512]
                            )
                            nc.sync.dma_start(dbg[2 + i], db[:])
                        for i in range(2):
                            db = opool.tile([P, 512], f32, tag="ob", name="db")
                            nc.vector.tensor_copy(
                                db[:], ex[:, i * 512 : (i + 1) * 512]
                            )
                            nc.sync.dma_start(dbg[4 + i], db[:])
                    if start:
                        cur_pv[blk] = [
                            pvps.tile([P, 512], f32, tag=f"pv{h}", name=f"pv{h}")
                            for h in range(2)
                        ]
                    if pending:
                        emit_pv(*pending.pop(0))
                    pending.append((hp, tq, tk, ex, cur_pv[blk], start, stop))
                    if not _SKEW:
                        flush_pending()

                def att_chunk(hp, tq, tkq):
                    """4-tk chunk of a CHUNKED block; PV -> SBUF accum."""
                    blk = (hp, tq)
                    for i in range(4):
                        att_step(hp, tq, tkq * 4 + i, start=(i == 0), stop=(i == 3))
                    flush_pending()
                    pvp = cur_pv.pop(blk)
                    an, ad = acc_n[blk], acc_d[blk]
                    for h in range(2):
                        hb = slice(h * 64, (h + 1) * 64)
                        if tkq == 0:
                            nc.vector.tensor_copy(an[hb, :], pvp[h][64:128, :])
                            nc.vector.tensor_copy(ad[hb, :], pvp[h][0:64, :])
                        else:
                            nc.vector.tensor_add(
                                an[hb, :], an[hb, :], pvp[h][64:128, :]
                            )
                            nc.vector.tensor_add(
                                ad[hb, :], ad[hb, :], pvp[h][0:64, :]
                            )

                def norm_post(hp, tq):
                    if _DBG:
                        nc.gpsimd.dma_start(
                            dbg[8 + 4 * hp + tq], attn_q[hp][tq][:].bitcast(f32)
                        )

                def norm_chunked(hp, tq):
                    blk = (hp, tq)
                    rc2 = npool.tile([P, 512], f32, tag="rc2", name="rc2")
                    # single full-tile recip: the custom-DVE op is base-0 only
                    nc.vector.reciprocal_approx_fast(rc2[:], acc_d[blk][:])
                    for h in range(2):
                        hb = slice(h * 64, (h + 1) * 64)
                        nc.vector.tensor_mul(
                            attn_q[hp][tq][hb, :], acc_n[blk][hb, :], rc2[hb, :]
                        )
                    norm_post(hp, tq)

                def norm_dense(hp, tq, pvp):
                    if _DBG:
                        db = opool.tile([P, 512], f32, tag="ob", name="db")
                        nc.vector.tensor_copy(db[:], pvp[0][:])
                        nc.sync.dma_start(dbg[16 + 2 * (tq - 2) + hp], db[:])
                    for h in range(2):
                        rc = npool.tile([64, 512], f32, tag="rc", name="rc")
                        nc.vector.reciprocal_approx_fast(rc[:], pvp[h][0:64, :])
                        hb = h * 64
                        nc.vector.tensor_mul(
                            attn_q[hp][tq][hb : hb + 64, :],
                            pvp[h][64:128, :],
                            rc[:],
                        )
                    norm_post(hp, tq)

                def outproj_unit(b, tqc):
                    row = b * 4 + tqc
                    for d2 in range(2):
                        po = wkps.tile([P, 512], f32, tag="wk", name="po")
                        for hp in range(2):
                            nc.tensor.matmul(
                                po[:],
                                lhsT=attn_q[hp][b][:, tqc * P : (tqc + 1) * P],
                                rhs=wo_sb[:, hp, d2 * 512 : (d2 + 1) * 512],
                                start=(hp == 0),
                                stop=(hp == 1),
                            )
                        ob = opool.tile([P, 512], f32, tag="ob", name="ob")
                        if d2 == 0:
                            nc.vector.tensor_copy(ob[:], po[:])
                        else:
                            nc.scalar.copy(ob[:], po[:])
                        seng = nc.sync if d2 == 0 else nc.gpsimd
                        seng.dma_start(
                            out[row * P : (row + 1) * P, d2 * 512 : (d2 + 1) * 512],
                            ob[:],
                        )

                def drain_act(dst, ps):
                    nc.scalar.copy(dst, ps)

                def drain_dve(dst, ps):
                    nc.vector.tensor_copy(dst, ps)

                # ---- schedule ---------------------------------------
                # Phase A: per-quarter rounds [KV(j) Q(j) + chunks whose
                # (q, k-quarter) inputs exist].  Later x/cos/sin kicks are
                # emitted mid-round so their descriptors enter the DMA
                # rings only once the critical early transfers finish
                # (engine queues execute kicks in program order, gated by
                # the dep-waiting ops emitted before them).
                ROUND_CHUNKS = [
                    [(0, 0, 0), (1, 0, 0)],
                    [(0, 0, 1), (1, 0, 1), (0, 1, 0), (1, 1, 0)],
                    [(0, 0, 2), (1, 0, 2), (0, 1, 1), (1, 1, 1),
                     (0, 2, 0), (1, 2, 0)],
                    [(0, 0, 3), (1, 0, 3), (0, 1, 2), (1, 1, 2),
                     (0, 2, 1), (1, 2, 1), (0, 3, 0), (1, 3, 0)],
                ]
                for j in range(4):
                    xt = xts[j]
                    flush_pending()
                    ktmps = []
                    for hp in range(2):
                        kt = rpool.tile([P, 512], bf16, tag="kt", name="kt")
                        fm_chain(xt, kt, 2 * hp + 1, drain_act)
                        ktmps.append(kt)
                    for hp in range(2):
                        rope_k(ktmps[hp], hp, j)
                    # stage x quarter j+1 (ring WAR: emitted after the
                    # previous slot user's readers; kicks queue behind the
                    # dep-gated rope DMAs above)
                    if j + 1 < 4:
                        xtl = xpool.tile([P, 4, 512], bf16, tag="xtl", name="xtl")
                        xth = xpool.tile([P, 4, 512], bf16, tag="xth", name="xth")
                        nc.sync.dma_start(xtl[:], xT[j + 1, :, 0:4, :])
                        nc.gpsimd.dma_start(xth[:], xT[j + 1, :, 4:8, :])
                        xts.append((xtl, xth))
                    v_pair(xt, j, 0)
                    v_pair(xt, j, 1)
                    # stage cos/sin for the next quarter (scalar queue,
                    # behind the v drains); w_o before outproj needs it
                    if j + 1 < 4:
                        hsn = slice((j + 1) * 512, (j + 2) * 512)
                        nc.scalar.dma_start(cos_sb[:, hsn], cos2[:, hsn])
                        nc.scalar.dma_start(sin_sb[:, hsn], sin2[:, hsn])
                    if j == 2:
                        nc.scalar.dma_start(wo_sb[:], w_o[:])
                    for hp in range(2):
                        fm_chain(xt, q_q[hp][j], 2 * hp, drain_dve)
                    for hp in range(2):
                        rope(q_q[hp][j], j)
                    for hp, tq, tkq in ROUND_CHUNKS[j]:
                        att_chunk(hp, tq, tkq)
                        if tkq == 3:
                            norm_chunked(hp, tq)

                # Phase B: remaining chunks; outproj b interleaves once
                # both its norms are done; OP(3) at the tail.
                PHASE_B = [
                    (0, 1, 3), (1, 1, 3),
                    (0, 2, 2), (1, 2, 2), (0, 2, 3), (1, 2, 3),
                    (0, 3, 1), (1, 3, 1), (0, 3, 2), (1, 3, 2),
                    (0, 3, 3), (1, 3, 3),
                ]
                OP_AFTER = {
                    0: (0, 0), 1: (0, 1), 2: (0, 2), 3: (0, 3),
                    4: (1, 0), 5: (1, 1), 6: (1, 2), 7: (1, 3),
                    8: (2, 0), 9: (2, 1), 10: (2, 2), 11: (2, 3),
                }
                for i, (hp, tq, tkq) in enumerate(PHASE_B):
                    att_chunk(hp, tq, tkq)
                    if tkq == 3:
                        norm_chunked(hp, tq)
                    if i in OP_AFTER:
                        outproj_unit(*OP_AFTER[i])
                for tqc in range(4):
                    outproj_unit(3, tqc)

    nc.compile()
    return nc


def _get_module():
    if "nc" not in _CACHE:
        _CACHE["nc"] = _build_module()
    return _CACHE["nc"]


def make_in_maps(x, w_qkv, w_out):
    cos2, sin2 = _rope_tables_np()
    in_maps = []
    for c in range(NCORES):
        b, g = divmod(c, 4)
        q0 = 256 * g
        # column chunks: [q_hp0 | k_hp0 | q_hp1 | k_hp1]
        wqk_c = np.concatenate(
            [
                w_qkv[:, q0 : q0 + 128],
                w_qkv[:, 1024 + q0 : 1024 + q0 + 128],
                w_qkv[:, q0 + 128 : q0 + 256],
                w_qkv[:, 1024 + q0 + 128 : 1024 + q0 + 256],
            ],
            axis=1,
        )
        xt4 = np.ascontiguousarray(
            x[b].T.reshape(8, 128, 4, 512).transpose(2, 1, 0, 3)
        )
        wv_c = w_qkv[:, 2048 + q0 : 2048 + q0 + 256]
        in_maps.append(
            {
                "xT": xt4.astype(BF16),
                "w_qk": np.ascontiguousarray(
                    wqk_c.reshape(8, 128, 512).transpose(1, 0, 2)
                ).astype(BF16),
                "w_v": np.ascontiguousarray(
                    wv_c.reshape(8, 128, 256).transpose(1, 0, 2)
                ).astype(BF16),
                "w_o": np.ascontiguousarray(
                    w_out[q0 : q0 + 256, :].reshape(2, 128, 1024).transpose(1, 0, 2)
                ),
                "cos2": cos2.astype(BF16),
                "sin2": sin2.astype(BF16),
            }
        )
    return in_maps


def combine_outputs(results, b_out):
    out = np.empty((B, T, D), dtype=np.float32)
    for b in range(B):
        acc = results[4 * b]["out"].astype(np.float32).copy()
        for c in range(4 * b + 1, 4 * b + 4):
            acc += results[c]["out"]
        out[b] = acc + b_out[None, :]
    return out


def kernel(x, w_qkv, w_out, b_out, _trace=False, _tag=[0]):
    from concourse import bass_utils

    nc = _get_module()
    in_maps = make_in_maps(
        np.asarray(x, dtype=np.float32),
        np.asarray(w_qkv, dtype=np.float32),
        np.asarray(w_out, dtype=np.float32),
    )
    res = bass_utils.run_bass_kernel_spmd(
        nc, in_maps, core_ids=list(range(NCORES)), trace=_trace
    )
    if _trace:
        _CACHE["last_result"] = res
    return combine_outputs(res.results, np.asarray(b_out, dtype=np.float32))


# revision 37
# speedup vs baseline: 1.1416x; 1.0201x over previous
"""Multi-head self-attention (RoPE, eval-mode) Trainium2 Bass kernel.

Problem: B=2, T=2048, D=1024, H=16, d_head=64, fp32 I/O.

Sharding (8 cores): core c handles batch b=c//4 and the 4 heads
[4g, 4g+4) where g=c%4.  QKV/attention are head-local; the output
projection produces a per-core partial (contraction over this core's
256 head-dims) which the host sums across the 4 cores of each batch
and adds b_out.

v2 design (vs the two-phase baseline):
  - The ACT exp stream (4 heads * T^2 = 16.8M elems ~ 110us streaming)
    is the hard wall.  The kernel is restructured so exp starts ~16us
    in instead of ~81us: per-quarter rounds emit the K/V/Q projection
    chains interleaved with attention tk-chunks of the first four
    (hp, tq) blocks, which accumulate PV partials into SBUF so the two
    PSUM pv banks don't serialize in-flight blocks.
  - DMA priority: w_qkv chunk 0 + x quarter 0 are issued first and the
    rest in need order, so the first matmul fires at ~6us not ~22us.
  - Scores are issued as two concurrent K=64 row-group matmuls
    (head 0 rows 0:64, head 1 rows 64:128) instead of zero-padded
    K=128 - halves score cycles; RoPE applies in place on the stacked
    k tiles (kstack == stationary source).
  - Emission skews sc one tk ahead of pv so the exp stream stays dense
    across chunk/block boundaries; per-head norm frees pv banks early.
  - PSUM: sc 2x[128,1024] (4 banks) + pv0/pv1 (2) + work ring 2 = 8.
  - v is computed row-major and stored per head as [ones | v] 128-wide
    stationary tiles: each PV matmul yields softmax denominators
    (partitions 0:64) and attn^T (64:128) in one pass.
  - softmax skips max-subtraction (scores ~ N(0,1), exp safe in fp32)
    and normalizes with the fast DVE reciprocal.
"""

import ml_dtypes
import numpy as np

BF16 = ml_dtypes.bfloat16

B, T, D = 2, 2048, 1024
H = 16
DH = 64
NCORES = 8
P = 128

_CACHE = {}
_DBG = False  # debug build: DMA intermediates of block (0,2) to "dbg"
_SKEW = True  # emit sc one tk ahead of the pv flush (denser ACT stream)
_OPTAIL = False  # emit all outproj units at the tail (diagnostic)
_ALLCHUNK = False  # process dense blocks as 4-tk chunks too (diagnostic)


def _rope_tables_np():
    theta = 1.0 / (10000.0 ** (np.arange(0, DH, 2, dtype=np.float32) / DH))
    angles = np.outer(np.arange(T, dtype=np.float32), theta)  # (T, 32)
    angles = np.concatenate([angles, angles], axis=-1)  # (T, DH)
    cos = np.cos(angles).astype(np.float32)
    sin = np.sin(angles).astype(np.float32)
    cosT = np.ascontiguousarray(cos.T)  # (64, T)
    sinT = np.ascontiguousarray(sin.T)
    sinT_signed = np.concatenate([-sinT[0:32], sinT[32:64]], axis=0)
    cos2 = np.tile(cosT, (2, 1))  # (128, T)
    sin2 = np.tile(sinT_signed, (2, 1))
    return cos2, sin2


def _build_module():
    import concourse.mybir as mybir
    import concourse.tile as tile
    from concourse import bacc

    f32 = mybir.dt.float32
    f32r = mybir.dt.float32r
    bf16 = mybir.dt.bfloat16

    nc = bacc.Bacc("TRN2", target_bir_lowering=False, debug=False)
    xT = nc.dram_tensor("xT", [4, P, 8, 512], bf16, kind="ExternalInput")
    w_qk = nc.dram_tensor("w_qk", [P, 8, 512], bf16, kind="ExternalInput")
    w_v = nc.dram_tensor("w_v", [P, 8, 256], bf16, kind="ExternalInput")
    w_o = nc.dram_tensor("w_o", [P, 2, 1024], f32r, kind="ExternalInput")
    cos2 = nc.dram_tensor("cos2", [P, T], bf16, kind="ExternalInput")
    sin2 = nc.dram_tensor("sin2", [P, T], bf16, kind="ExternalInput")
    out = nc.dram_tensor("out", [T, D], bf16, kind="ExternalOutput")
    dbg = (
        nc.dram_tensor("dbg", [20, P, 512], f32, kind="ExternalOutput")
        if _DBG
        else None
    )

    Exp = mybir.ActivationFunctionType.Exp

    # Blocks in completion order.  The first CHUNKED ones accumulate PV
    # into SBUF in 4-tk chunks paced by quarter availability; the rest
    # run dense (16 tk straight, PV resident in PSUM).
    CHUNKED = [(hp, tq) for tq in range(4) for hp in range(2)]

    with tile.TileContext(nc) as tc:
        with tc.tile_pool(name="persist", bufs=1) as persist:
            wqk_sb = [
                persist.tile([P, 4, 512], bf16, tag=f"wqk{i}", name=f"wqk{i}")
                for i in range(2)
            ]
            wv_sb = persist.tile([P, 8, 256], bf16)
            wo_sb = persist.tile([P, 2, 1024], f32r)
            cos_sb = persist.tile([P, T], bf16)
            sin_sb = persist.tile([P, T], bf16)
            # roped q / stacked roped k, two heads per 128-partition tile
            q_q = [
                [persist.tile([P, 512], bf16, tag=f"q{hp}_{t}", name=f"q{hp}_{t}")
                 for t in range(4)]
                for hp in range(2)
            ]
            # zero-padded roped k per head (head h real rows h*64:h*64+64,
            # other half zero) - K=128 score matmuls need no row groups
            kpad = [
                [
                    [persist.tile([P, 512], bf16, tag=f"k{hp}{h}_{t}",
                                  name=f"k{hp}{h}_{t}")
                     for t in range(4)]
                    for h in range(2)
                ]
                for hp in range(2)
            ]
            # per (tk-tile, head): [ones | v] stationary 128x128
            vaug = persist.tile([P, 16, 4, P], bf16)
            attn_q = [
                [persist.tile([P, 512], f32r, tag=f"at{hp}_{b}", name=f"at{hp}_{b}")
                 for b in range(4)]
                for hp in range(2)
            ]
            # SBUF PV accumulators for the chunked blocks, partition-
            # aligned with attn_q: acc_n rows h*64:(h+1)*64 = head h
            # numerator (attn^T), acc_d same layout for denominators.
            acc_n = {
                blk: persist.tile([P, 512], f32, tag=f"an{blk[0]}{blk[1]}",
                                  name=f"an{blk[0]}{blk[1]}")
                for blk in CHUNKED
            }
            acc_d = {
                blk: persist.tile([P, 512], f32, tag=f"ad{blk[0]}{blk[1]}",
                                  name=f"ad{blk[0]}{blk[1]}")
                for blk in CHUNKED
            }

            with (
                tc.tile_pool(name="xt", bufs=2) as xpool,
                tc.tile_pool(name="rope", bufs=2) as rpool,
                tc.tile_pool(name="expp", bufs=4) as epool,
                tc.tile_pool(name="ob", bufs=4) as opool,
                tc.tile_pool(name="norm", bufs=1) as npool,
                tc.tile_pool(name="sc_ps", bufs=2, space="PSUM") as scps,
                tc.tile_pool(name="pv_ps", bufs=1, space="PSUM") as pvps,
                tc.tile_pool(name="wk_ps", bufs=2, space="PSUM") as wkps,
            ):
                # ---- input DMAs in priority order --------------------
                nc.sync.dma_start(wqk_sb[0][:], w_qk[:, 0:4, :])
                nc.gpsimd.dma_start(wqk_sb[1][:], w_qk[:, 4:8, :])
                xts = []
                xtl = xpool.tile([P, 4, 512], bf16, tag="xtl", name="xtl")
                xth = xpool.tile([P, 4, 512], bf16, tag="xth", name="xth")
                nc.sync.dma_start(xtl[:], xT[0, :, 0:4, :])
                nc.gpsimd.dma_start(xth[:], xT[0, :, 4:8, :])
                xts.append((xtl, xth))
                nc.gpsimd.memset(vaug[:, :, :, 0:64], 1.0)
                for hp in range(2):
                    for t in range(4):
                        nc.gpsimd.memset(kpad[hp][0][t][64:128, :], 0.0)
                        nc.gpsimd.memset(kpad[hp][1][t][0:64, :], 0.0)

                # ---- unit emitters ----------------------------------
                def fm_chain(xt, dst, cc, drain):
                    """q or stacked-k feature-major chain -> dst (SBUF).
                    cc: column chunk in wqk ([q_hp0 | k_hp0 | q_hp1 | k_hp1])."""
                    ps = wkps.tile([P, 512], f32, tag="wk", name="wk")
                    for dc in range(8):
                        nc.tensor.matmul(
                            ps[:],
                            lhsT=wqk_sb[dc // 4][:, dc % 4, cc * P : (cc + 1) * P],
                            rhs=xt[dc // 4][:, dc % 4, :],
                            start=(dc == 0),
                            stop=(dc == 7),
                        )
                    drain(dst[:], ps[:])

                def v_pair(xt, tq, half):
                    """two T-128 blocks of v for all 4 heads -> vaug."""
                    psv = wkps.tile([P, 512], f32, tag="wk", name="wkv")
                    for t4 in (2 * half, 2 * half + 1):
                        off = (t4 % 2) * 256
                        for dc in range(8):
                            nc.tensor.matmul(
                                psv[:, off : off + 256],
                                lhsT=xt[dc // 4][:, dc % 4, t4 * P : (t4 + 1) * P],
                                rhs=wv_sb[:, dc, :],
                                start=(dc == 0),
                                stop=(dc == 7),
                            )
                    tki = tq * 4 + 2 * half
                    nc.scalar.copy(
                        vaug[:, tki : tki + 2, :, 64:128],
                        psv.rearrange("p (t h e) -> p t h e", t=2, e=64),
                    )

                first_rope = [True]

                def rope_mats(base, tq):
                    hs = slice(tq * 512, (tq + 1) * 512)
                    rot = rpool.tile([P, 512], bf16, tag="rot", name="rot")
                    for blk in range(4):
                        s = (blk ^ 1) * 32
                        eng = nc.sync if blk % 2 == 0 else nc.gpsimd
                        eng.dma_start(
                            rot[blk * 32 : (blk + 1) * 32, :],
                            base[s : s + 32, :],
                        )
                        if first_rope[0]:
                            # second-wave input kicks: queued behind the
                            # dep-gated rot DMA above, so they enter the
                            # DMA rings only once wqk/x0 are ~done
                            first_rope[0] = False
                            hs0 = slice(0, 512)
                            nc.sync.dma_start(cos_sb[:, hs0], cos2[:, hs0])
                            nc.sync.dma_start(sin_sb[:, hs0], sin2[:, hs0])
                            nc.sync.dma_start(wv_sb[:], w_v[:])
                    t1 = rpool.tile([P, 512], bf16, tag="t1", name="t1")
                    nc.vector.tensor_mul(t1[:], base[:], cos_sb[:, hs])
                    nc.vector.tensor_mul(rot[:], rot[:], sin_sb[:, hs])
                    return t1, rot

                def rope(base, tq):
                    """RoPE in place on a [128,512] stacked bf16 tile."""
                    t1, rot = rope_mats(base, tq)
                    nc.vector.tensor_add(base[:], t1[:], rot[:])

                def rope_k(ktmp, hp, tq):
                    """RoPE stacked k into the per-head zero-padded tiles."""
                    t1, rot = rope_mats(ktmp, tq)
                    nc.vector.tensor_add(
                        kpad[hp][0][tq][0:64, :], t1[0:64, :], rot[0:64, :]
                    )
                    nc.vector.tensor_add(
                        kpad[hp][1][tq][64:128, :], t1[64:128, :], rot[64:128, :]
                    )

                # attention step machinery: sc is emitted one tk ahead of
                # the pv flush so the ACT exp stream stays dense.
                pending = []  # list of (hp, tq, tk, ex, pv_pair, start, stop)

                def flush_pending():
                    while pending:
                        emit_pv(*pending.pop(0))

                def emit_pv(hp, tq, tk, ex, pvp, start, stop):
                    for h in range(2):
                        nc.tensor.matmul(
                            pvp[h][:],
                            lhsT=vaug[:, tk, hp * 2 + h, :],
                            rhs=ex[:, h * 512 : (h + 1) * 512],
                            start=start,
                            stop=stop,
                        )
                    if stop:
                        accum_chunk(hp, tq, tk // 4, pvp)

                cur_pv = {}  # blk -> [pv0, pv1] while a chunk is in flight

                def att_step(hp, tq, tk, start, stop):
                    blk = (hp, tq)
                    sc = scps.tile([P, 1024], f32, tag="sc", name="sc")
                    ko = (tk % 4) * P
                    for h in range(2):
                        nc.tensor.matmul(
                            sc[:, h * 512 : (h + 1) * 512],
                            lhsT=kpad[hp][h][tk // 4][:, ko : ko + P],
                            rhs=q_q[hp][tq][:],
                            start=True,
                            stop=True,
                        )
                    ex = epool.tile([P, 1024], bf16, tag="e", name="e")
                    nc.scalar.activation(ex[:], sc[:], Exp, scale=0.125)
                    if _DBG and (hp, tq, tk) == (1, 2, 0):
                        for i, tsrc in ((0, q_q[1][2]), (1, kpad[1][0][0])):
                            db = opool.tile([P, 512], f32, tag="ob", name="db")
                            nc.vector.tensor_copy(db[:], tsrc[:])
                            nc.sync.dma_start(dbg[i], db[:])
                        for i in range(2):
                            db = opool.tile([P, 512], f32, tag="ob", name="db")
                            nc.vector.tensor_copy(
                                db[:], sc[:, i * 512 : (i + 1) * 512]
                            )
                            nc.sync.dma_start(dbg[2 + i], db[:])
                        for i in range(2):
                            db = opool.tile([P, 512], f32, tag="ob", name="db")
                            nc.vector.tensor_copy(
                                db[:], ex[:, i * 512 : (i + 1) * 512]
                            )
                            nc.sync.dma_start(dbg[4 + i], db[:])
                    if start:
                        cur_pv[blk] = [
                            pvps.tile([P, 512], f32, tag=f"pv{h}", name=f"pv{h}")
                            for h in range(2)
                        ]
                    if pending:
                        emit_pv(*pending.pop(0))
                    pending.append((hp, tq, tk, ex, cur_pv[blk], start, stop))
                    if not _SKEW:
                        flush_pending()

                def accum_chunk(hp, tq, tkq, pvp):
                    """fold a finished 4-tk pv chunk into the SBUF accum;
                    called from emit_pv when the stop pv lands, so the skew
                    pipeline never breaks at chunk boundaries."""
                    blk = (hp, tq)
                    an, ad = acc_n[blk], acc_d[blk]
                    for h in range(2):
                        hb = slice(h * 64, (h + 1) * 64)
                        if tkq == 0:
                            nc.vector.tensor_copy(an[hb, :], pvp[h][64:128, :])
                            nc.vector.tensor_copy(ad[hb, :], pvp[h][0:64, :])
                        else:
                            nc.vector.tensor_add(
                                an[hb, :], an[hb, :], pvp[h][64:128, :]
                            )
                            nc.vector.tensor_add(
                                ad[hb, :], ad[hb, :], pvp[h][0:64, :]
                            )
                    if tkq == 3:
                        norm_chunked(hp, tq)

                def att_chunk(hp, tq, tkq):
                    """4-tk chunk of a CHUNKED block; PV -> SBUF accum."""
                    blk = (hp, tq)
                    for i in range(4):
                        att_step(hp, tq, tkq * 4 + i, start=(i == 0), stop=(i == 3))
                    cur_pv.pop(blk)

                def norm_post(hp, tq):
                    if _DBG:
                        nc.gpsimd.dma_start(
                            dbg[8 + 4 * hp + tq], attn_q[hp][tq][:].bitcast(f32)
                        )

                def norm_chunked(hp, tq):
                    blk = (hp, tq)
                    rc2 = npool.tile([P, 512], f32, tag="rc2", name="rc2")
                    # single full-tile recip: the custom-DVE op is base-0 only
                    nc.vector.reciprocal_approx_fast(rc2[:], acc_d[blk][:])
                    for h in range(2):
                        hb = slice(h * 64, (h + 1) * 64)
                        nc.vector.tensor_mul(
                            attn_q[hp][tq][hb, :], acc_n[blk][hb, :], rc2[hb, :]
                        )
                    norm_post(hp, tq)

                def norm_dense(hp, tq, pvp):
                    if _DBG:
                        db = opool.tile([P, 512], f32, tag="ob", name="db")
                        nc.vector.tensor_copy(db[:], pvp[0][:])
                        nc.sync.dma_start(dbg[16 + 2 * (tq - 2) + hp], db[:])
                    for h in range(2):
                        rc = npool.tile([64, 512], f32, tag="rc", name="rc")
                        nc.vector.reciprocal_approx_fast(rc[:], pvp[h][0:64, :])
                        hb = h * 64
                        nc.vector.tensor_mul(
                            attn_q[hp][tq][hb : hb + 64, :],
                            pvp[h][64:128, :],
                            rc[:],
                        )
                    norm_post(hp, tq)

                def outproj_unit(b, tqc):
                    row = b * 4 + tqc
                    for d2 in range(2):
                        po = wkps.tile([P, 512], f32, tag="wk", name="po")
                        for hp in range(2):
                            nc.tensor.matmul(
                                po[:],
                                lhsT=attn_q[hp][b][:, tqc * P : (tqc + 1) * P],
                                rhs=wo_sb[:, hp, d2 * 512 : (d2 + 1) * 512],
                                start=(hp == 0),
                                stop=(hp == 1),
                            )
                        ob = opool.tile([P, 512], bf16, tag="ob", name="ob")
                        if d2 == 0:
                            nc.vector.tensor_copy(ob[:], po[:])
                        else:
                            nc.scalar.copy(ob[:], po[:])
                        seng = nc.sync if d2 == 0 else nc.gpsimd
                        seng.dma_start(
                            out[row * P : (row + 1) * P, d2 * 512 : (d2 + 1) * 512],
                            ob[:],
                        )

                def drain_act(dst, ps):
                    nc.scalar.copy(dst, ps)

                def drain_dve(dst, ps):
                    nc.vector.tensor_copy(dst, ps)

                # ---- schedule ---------------------------------------
                # Phase A: per-quarter rounds [KV(j) Q(j) + chunks whose
                # (q, k-quarter) inputs exist].  Later x/cos/sin kicks are
                # emitted mid-round so their descriptors enter the DMA
                # rings only once the critical early transfers finish
                # (engine queues execute kicks in program order, gated by
                # the dep-waiting ops emitted before them).
                ROUND_CHUNKS = [
                    [(0, 0, 0), (1, 0, 0)],
                    [(0, 0, 1), (1, 0, 1), (0, 1, 0), (1, 1, 0)],
                    [(0, 0, 2), (1, 0, 2), (0, 1, 1), (1, 1, 1),
                     (0, 2, 0), (1, 2, 0)],
                    [(0, 0, 3), (1, 0, 3), (0, 1, 2), (1, 1, 2),
                     (0, 2, 1), (1, 2, 1), (0, 3, 0), (1, 3, 0)],
                ]
                for j in range(4):
                    xt = xts[j]
                    flush_pending()
                    ktmps = []
                    for hp in range(2):
                        kt = rpool.tile([P, 512], bf16, tag="kt", name="kt")
                        fm_chain(xt, kt, 2 * hp + 1, drain_act)
                        ktmps.append(kt)
                    for hp in range(2):
                        rope_k(ktmps[hp], hp, j)
                    # stage x quarter j+1 (ring WAR: emitted after the
                    # previous slot user's readers; kicks queue behind the
                    # dep-gated rope DMAs above)
                    if j + 1 < 4:
                        xtl = xpool.tile([P, 4, 512], bf16, tag="xtl", name="xtl")
                        xth = xpool.tile([P, 4, 512], bf16, tag="xth", name="xth")
                        nc.sync.dma_start(xtl[:], xT[j + 1, :, 0:4, :])
                        nc.gpsimd.dma_start(xth[:], xT[j + 1, :, 4:8, :])
                        xts.append((xtl, xth))
                    v_pair(xt, j, 0)
                    v_pair(xt, j, 1)
                    # stage cos/sin for the next quarter (scalar queue,
                    # behind the v drains); w_o before outproj needs it
                    if j + 1 < 4:
                        hsn = slice((j + 1) * 512, (j + 2) * 512)
                        nc.scalar.dma_start(cos_sb[:, hsn], cos2[:, hsn])
                        nc.scalar.dma_start(sin_sb[:, hsn], sin2[:, hsn])
                    if j == 2:
                        nc.scalar.dma_start(wo_sb[:], w_o[:])
                    for hp in range(2):
                        fm_chain(xt, q_q[hp][j], 2 * hp, drain_dve)
                    for hp in range(2):
                        rope(q_q[hp][j], j)
                    for hp, tq, tkq in ROUND_CHUNKS[j]:
                        att_chunk(hp, tq, tkq)

                # Phase B: remaining chunks; outproj b interleaves once
                # both its norms are done; OP(3) at the tail.
                PHASE_B = [
                    (0, 1, 3), (1, 1, 3),
                    (0, 2, 2), (1, 2, 2), (0, 2, 3), (1, 2, 3),
                    (0, 3, 1), (1, 3, 1), (0, 3, 2), (1, 3, 2),
                    (0, 3, 3), (1, 3, 3),
                ]
                OP_AFTER = {
                    0: (0, 0), 1: (0, 1), 2: (0, 2), 3: (0, 3),
                    4: (1, 0), 5: (1, 1), 6: (1, 2), 7: (1, 3),
                    8: (2, 0), 9: (2, 1), 10: (2, 2), 11: (2, 3),
                }
                for i, (hp, tq, tkq) in enumerate(PHASE_B):
                    att_chunk(hp, tq, tkq)
                    if i in OP_AFTER:
                        outproj_unit(*OP_AFTER[i])
                flush_pending()
                for tqc in range(4):
                    outproj_unit(3, tqc)

    nc.compile()
    return nc


def _get_module():
    if "nc" not in _CACHE:
        _CACHE["nc"] = _build_module()
    return _CACHE["nc"]


def make_in_maps(x, w_qkv, w_out):
    cos2, sin2 = _rope_tables_np()
    in_maps = []
    for c in range(NCORES):
        b, g = divmod(c, 4)
        q0 = 256 * g
        # column chunks: [q_hp0 | k_hp0 | q_hp1 | k_hp1]
        wqk_c = np.concatenate(
            [
                w_qkv[:, q0 : q0 + 128],
                w_qkv[:, 1024 + q0 : 1024 + q0 + 128],
                w_qkv[:, q0 + 128 : q0 + 256],
                w_qkv[:, 1024 + q0 + 128 : 1024 + q0 + 256],
            ],
            axis=1,
        )
        xt4 = np.ascontiguousarray(
            x[b].T.reshape(8, 128, 4, 512).transpose(2, 1, 0, 3)
        )
        wv_c = w_qkv[:, 2048 + q0 : 2048 + q0 + 256]
        in_maps.append(
            {
                "xT": xt4.astype(BF16),
                "w_qk": np.ascontiguousarray(
                    wqk_c.reshape(8, 128, 512).transpose(1, 0, 2)
                ).astype(BF16),
                "w_v": np.ascontiguousarray(
                    wv_c.reshape(8, 128, 256).transpose(1, 0, 2)
                ).astype(BF16),
                "w_o": np.ascontiguousarray(
                    w_out[q0 : q0 + 256, :].reshape(2, 128, 1024).transpose(1, 0, 2)
                ),
                "cos2": cos2.astype(BF16),
                "sin2": sin2.astype(BF16),
            }
        )
    return in_maps


def combine_outputs(results, b_out):
    out = np.empty((B, T, D), dtype=np.float32)
    for b in range(B):
        acc = results[4 * b]["out"].astype(np.float32)
        for c in range(4 * b + 1, 4 * b + 4):
            acc += results[c]["out"].astype(np.float32)
        out[b] = acc + b_out[None, :]
    return out


def kernel(x, w_qkv, w_out, b_out, _trace=False, _tag=[0]):
    from concourse import bass_utils

    nc = _get_module()
    in_maps = make_in_maps(
        np.asarray(x, dtype=np.float32),
        np.asarray(w_qkv, dtype=np.float32),
        np.asarray(w_out, dtype=np.float32),
    )
    res = bass_utils.run_bass_kernel_spmd(
        nc, in_maps, core_ids=list(range(NCORES)), trace=_trace
    )
    if _trace:
        _CACHE["last_result"] = res
    return combine_outputs(res.results, np.asarray(b_out, dtype=np.float32))
